# revision 1
# baseline (speedup 1.0000x reference)
"""GCN(GAT)x2 + LSTM + LN + dense on 8 Trainium2 NeuronCores (Bass/Tile).

Sharding: data-parallel over the B*T=48 graph replicas (6 per core, as 3
pair-tables); the layer1->layer2 scramble is an AllGather within each
4-core b-group; the LSTM stage consumes the core-local layer2 output
(the block assignment makes that transition local). All permutations /
sparse gathers are host-baked int16 index arrays driven through the
dma_gather (embedding-gather) DMA instruction against per-graph row
tables; the per-dst padded slot layout turns the segment softmax +
weighted aggregation into dense per-partition vector ops.
"""

import os
import sys
import numpy as np
import ml_dtypes

BF16 = ml_dtypes.bfloat16

N = 2000
NPAD = 2048
T = 24
B = 2
NX = 8
H = 64
E = 32000
PL = 12
NB = 16          # dst blocks of 128 pi-positions
DUM = 2048       # dummy table row index
TROWS = 2049     # rows per graph in a pair table
NEG = -1e9
LN_EPS = 1e-5
NCORES = 8
SEQ = 500        # LSTM sequences per core


def _ensure_hook():
    """Register the NTFF profile hook if the boot didn't (enables traces)."""
    try:
        from antenv.axon_hooks import (get_axon_ntff_profile_hook,
                                       set_axon_ntff_profile_hook)
        if get_axon_ntff_profile_hook() is None:
            from trn_agent_boot.trn_boot import _ntff_profile_via_ctypes
            so = "/opt/axon/libaxon_pjrt.so"
            if os.path.exists(so):
                set_axon_ntff_profile_hook(_ntff_profile_via_ctypes(so))
    except Exception:
        pass


# ---------------------------------------------------------------------------
# Host-side prep
# ---------------------------------------------------------------------------

def fmt_idx(flat):
    flat = np.asarray(flat)
    assert len(flat) % 128 == 0
    fmt = flat.reshape(-1, 16).T.astype(np.int16)
    return np.tile(fmt, (8, 1))


def prep_structure(edge_src, edge_dst):
    src = np.asarray(edge_src, np.int64)
    dst = np.asarray(edge_dst, np.int64)
    deg = np.bincount(dst, minlength=N)
    pi = np.argsort(deg, kind="stable")
    pi_inv = np.empty(N, np.int64)
    pi_inv[pi] = np.arange(N)

    dpos = pi_inv[dst]
    order = np.argsort(dpos, kind="stable")
    src_s = src[order]
    dpos_s = dpos[order]
    cnt = np.bincount(dpos_s, minlength=NPAD)
    dmax = [int(max(1, cnt[b * 128:(b + 1) * 128].max())) for b in range(NB)]
    coloff = np.concatenate([[0], np.cumsum(dmax)]).astype(np.int64)
    ncol = int(coloff[-1])

    starts = np.concatenate([[0], np.cumsum(cnt)]).astype(np.int64)
    slot = np.full((ncol, 128), DUM, np.int64)
    b_of = dpos_s // 128
    p_of = dpos_s % 128
    jw = np.arange(E) - starts[dpos_s]
    slot[coloff[b_of] + jw, p_of] = pi_inv[src_s]
    eidx = fmt_idx(slot.reshape(-1))

    per_t = []
    for t in range(T):
        nl = np.arange(SEQ)
        flat_i = nl * T + t
        lrow = (flat_i // N) * TROWS + pi_inv[flat_i % N]
        lrow = np.concatenate([lrow, np.full(12, -1, np.int64)])
        per_t.append(fmt_idx(lrow))  # [128, 32]
    lstmidx = np.ascontiguousarray(np.stack(per_t, axis=1))  # [128, 24, 32]

    return dict(pi=pi, pi_inv=pi_inv, dmax=dmax, coloff=coloff, ncol=ncol,
                eidx=eidx, lstmidx=lstmidx)


def prep_core(st, core):
    pi, pi_inv = st["pi"], st["pi_inv"]
    c = core % 4
    x2rows = np.full((6, NPAD), DUM, np.int64)
    pmask = np.zeros((6, NPAD), np.float32)
    ppos = np.arange(N)
    for lg in range(6):
        t2 = 6 * c + lg
        flat_i = pi[ppos] * T + t2
        tp, npr = flat_i // N, flat_i % N
        x2rows[lg, :N] = (tp // 6 * 3 + (tp % 6) // 2) * TROWS + pi_inv[npr]
        pmask[lg, :N] = ((tp % 6) % 2).astype(np.float32)
    x2idx = np.stack([fmt_idx(x2rows[lg]) for lg in range(6)], axis=1)
    return dict(x2idx=np.ascontiguousarray(x2idx.astype(np.int16)),
                pmask=pmask.astype(np.uint8))


def prep_weights(p):
    f32 = np.float32
    w = {}
    w["fcW"] = np.asarray(p["fc_W"], f32)
    w["fcb"] = np.asarray(p["fc_b"], f32).reshape(64, 1)
    w["W1"] = np.asarray(p["g1_W"], f32)
    w["W2"] = np.asarray(p["g2_W"], f32).astype(BF16)
    w["a1s"] = np.asarray(p["g1_asrc"], f32).reshape(1, 64)
    w["a1d"] = np.asarray(p["g1_adst"], f32).reshape(1, 64)
    w["a2s"] = np.asarray(p["g2_asrc"], f32).reshape(1, 64)
    w["a2d"] = np.asarray(p["g2_adst"], f32).reshape(1, 64)
    w["b1"] = np.asarray(p["g1_b"], f32).reshape(1, 64)
    w["b2"] = np.asarray(p["g2_b"], f32).reshape(1, 64)
    Wih = np.asarray(p["lstm_Wih"], f32)
    Whh = np.asarray(p["lstm_Whh"], f32)
    w["LL"] = np.vstack([Whh.T, Wih.T]).astype(BF16)  # K rows: 0:64=h, 64:128=x
    bihh = np.asarray(p["lstm_bih"], f32) + np.asarray(p["lstm_bhh"], f32)
    w["lstmb"] = np.ascontiguousarray(bihh.reshape(4, 64).T)  # [64,4] i,f,g,o
    w["lng"] = np.asarray(p["ln_g"], f32).reshape(64, 1)
    w["lnb"] = np.asarray(p["ln_b"], f32).reshape(64, 1)
    w["dW"] = np.asarray(p["dense_W"], f32)
    w["db"] = np.asarray(p["dense_b"], f32).reshape(1, 12)
    dum = np.zeros((1, 256), f32)
    dum[0, 64] = NEG
    dum[0, 192] = NEG
    w["dumrow"] = dum.astype(BF16)
    return w


def prep_xT(p, st, core):
    x = np.asarray(p["x"], np.float32)
    b, c = core // 4, core % 4
    pi = st["pi"]
    xT = np.zeros((NX, 6 * NPAD), np.float32)
    for lg in range(6):
        xT[:, lg * NPAD: lg * NPAD + N] = x[b, pi, 6 * c + lg, :].T
    return np.ascontiguousarray(xT)


# ---------------------------------------------------------------------------
# Bass program
# ---------------------------------------------------------------------------

def build_program(meta, stop=None):
    stop = stop or os.environ.get("K_STOP", "full")
    import concourse.bass as bass
    import concourse.mybir as mybir
    import concourse.tile as tile
    from concourse import bacc

    dt = mybir.dt
    Alu = mybir.AluOpType
    Act = mybir.ActivationFunctionType
    AX = mybir.AxisListType
    dmax, coloff, ncol = meta["dmax"], meta["coloff"], meta["ncol"]

    nc = bacc.Bacc("TRN2", target_bir_lowering=False, debug=False,
                   num_devices=NCORES)

    def din(name, shape, dty):
        return nc.dram_tensor(name, list(shape), dty, kind="ExternalInput")

    xT_d = din("xT", (NX, 6 * NPAD), dt.float32)
    eidx_d = din("eidx", (128, ncol * 8), dt.int16)
    x2idx_d = din("x2idx", (128, 6, 128), dt.int16)
    lstmidx_d = din("lstmidx", (128, T, 32), dt.int16)
    pmask_d = din("pmask", (6, NPAD), dt.uint8)
    fcW_d = din("fcW", (NX, 64), dt.float32)
    fcb_d = din("fcb", (64, 1), dt.float32)
    W1_d = din("W1", (64, 64), dt.float32)
    W2_d = din("W2", (64, 64), dt.bfloat16)
    a1s_d = din("a1s", (1, 64), dt.float32)
    a1d_d = din("a1d", (1, 64), dt.float32)
    a2s_d = din("a2s", (1, 64), dt.float32)
    a2d_d = din("a2d", (1, 64), dt.float32)
    b1_d = din("b1", (1, 64), dt.float32)
    b2_d = din("b2", (1, 64), dt.float32)
    LL_d = din("LL", (128, 256), dt.bfloat16)
    lstmb_d = din("lstmb", (64, 4), dt.float32)
    lng_d = din("lng", (64, 1), dt.float32)
    lnb_d = din("lnb", (64, 1), dt.float32)
    dW_d = din("dW", (64, 12), dt.float32)
    db_d = din("db", (1, 12), dt.float32)
    dum_d = din("dumrow", (1, 256), dt.bfloat16)
    out_d = nc.dram_tensor("out", [SEQ, 12], dt.float32, kind="ExternalOutput")

    def bcast_ap(dram_t, parts, free):
        """Partition-broadcast read AP of a [1, free] DRAM tensor."""
        ap = dram_t if isinstance(dram_t, bass.AP) else dram_t.ap()
        return bass.AP(tensor=ap.tensor, offset=ap.offset,
                       ap=[[0, parts]] + [list(x) for x in ap.ap[1:]])

    def blast(ap_, n):
        """Append a step-0 dim of size n to an AP (free-dim broadcast)."""
        return bass.AP(tensor=ap_.tensor, offset=ap_.offset,
                       ap=list(ap_.ap) + [[0, n]])

    with tile.TileContext(nc) as tc:
        import contextlib
        ctx = contextlib.ExitStack()
        consts = ctx.enter_context(tc.tile_pool(name="consts", bufs=1))
        dramp = ctx.enter_context(tc.tile_pool(name="dramp", bufs=1,
                                               space="DRAM"))
        ps_h = ctx.enter_context(tc.tile_pool(name="ps_h", bufs=2,
                                              space="PSUM"))
        ps_misc = ctx.enter_context(tc.tile_pool(name="ps_misc", bufs=1,
                                                 space="PSUM"))
        ps_z = ctx.enter_context(tc.tile_pool(name="ps_z", bufs=4,
                                              space="PSUM"))
        rows_p = ctx.enter_context(tc.tile_pool(name="rows", bufs=3))
        msg_p = ctx.enter_context(tc.tile_pool(name="msg", bufs=2))
        wk_p = ctx.enter_context(tc.tile_pool(name="wk", bufs=6))
        col_p = ctx.enter_context(tc.tile_pool(name="col", bufs=8))
        edt_p = ctx.enter_context(tc.tile_pool(name="edt", bufs=4))
        wm_p = ctx.enter_context(tc.tile_pool(name="wm", bufs=3))
        agg_p = ctx.enter_context(tc.tile_pool(name="agg", bufs=6))

        # ---- constants into SBUF ----
        def load(name, shape, dty, src_ap):
            t = consts.tile(shape, dty, name=name)
            nc.sync.dma_start(out=t[:], in_=src_ap)
            return t

        eidx_s = load("eidx_s", [128, ncol * 8], dt.int16, eidx_d[:])
        x2idx_s = load("x2idx_s", [128, 6, 128], dt.int16, x2idx_d[:])
        lidx_s = load("lidx_s", [128, T, 32], dt.int16, lstmidx_d[:])
        fcW_s = load("fcW_s", [NX, 64], dt.float32, fcW_d[:])
        fcb_s = load("fcb_s", [64, 1], dt.float32, fcb_d[:])
        W1_s = load("W1_s", [64, 64], dt.float32, W1_d[:])
        W2_s = load("W2_s", [64, 64], dt.bfloat16, W2_d[:])
        LL_s = load("LL_s", [128, 256], dt.bfloat16, LL_d[:])
        dW_s = load("dW_s", [64, 12], dt.float32, dW_d[:])
        lstmb_s = load("lstmb_s", [64, 4], dt.float32, lstmb_d[:])
        lng_s = load("lng_s", [64, 1], dt.float32, lng_d[:])
        lnb_s = load("lnb_s", [64, 1], dt.float32, lnb_d[:])
        # partition-broadcast constants
        a_R = {}
        for nm, d_ in (("a1s", a1s_d), ("a1d", a1d_d), ("a2s", a2s_d),
                       ("a2d", a2d_d), ("b1", b1_d), ("b2", b2_d)):
            t = consts.tile([128, 64], dt.float32, name=nm + "R")
            nc.gpsimd.dma_start(out=t[:], in_=bcast_ap(d_, 128, 64))
            a_R[nm] = t
        dbR = consts.tile([128, 12], dt.float32, name="dbR")
        nc.gpsimd.dma_start(out=dbR[:], in_=bcast_ap(db_d, 128, 12))
        onesrow = consts.tile([1, 64], dt.float32, name="onesrow")
        nc.vector.memset(onesrow[:], 1.0)
        onescol = consts.tile([128, 1], dt.float32, name="onescol")
        nc.vector.memset(onescol[:], 1.0)
        epsT = consts.tile([1, 1], dt.float32, name="epsT")
        nc.vector.memset(epsT[:], LN_EPS)

        # ---- DRAM tables ----
        htab = {1: dramp.tile([3, TROWS, 256], dt.bfloat16, name="htab1"),
                2: dramp.tile([3, TROWS, 256], dt.bfloat16, name="htab2")}
        l1out = dramp.tile([3, TROWS, 256], dt.bfloat16, name="l1out")
        agtab = dramp.tile([12 * TROWS, 256], dt.bfloat16, name="agtab")
        lstmtab = dramp.tile([6 * TROWS, 128], dt.bfloat16, name="lstmtab")

        # ---- FC: hfcT [64, 12288] = fcW.T @ xT + fcb ----
        fc_ctx = contextlib.ExitStack()
        hfc_p = fc_ctx.enter_context(tc.tile_pool(name="hfc", bufs=1))
        hfcT = hfc_p.tile([64, 6 * NPAD], dt.float32, name="hfcT")
        with tc.tile_pool(name="xtp", bufs=1) as xt_p:
            xT_s = xt_p.tile([NX, 6 * NPAD], dt.float32, name="xT_s")
            nc.sync.dma_start(out=xT_s[:], in_=xT_d[:])
            for chix in range(6 * NPAD // 512):
                sl = slice(chix * 512, (chix + 1) * 512)
                ps = ps_z.tile([64, 512], dt.float32, space="PSUM",
                               tag="pslstm", name="psfc")
                nc.tensor.matmul(ps[:], lhsT=fcW_s[:], rhs=xT_s[:, sl],
                                 start=True, stop=True)
                nc.vector.tensor_scalar_add(hfcT[:, sl], ps[:], fcb_s[:])

        scr64 = wk_p  # alias for readability

        def table_build(layer, pair, lhsT_of):
            """Build pair h-table (rows [128,256]) + return ed tiles."""
            Wmat = W1_s if layer == 1 else W2_s
            a_s = a_R["a1s"] if layer == 1 else a_R["a2s"]
            a_d = a_R["a1d"] if layer == 1 else a_R["a2d"]
            edts = []
            for par in (0, 1):
                edts.append(edt_p.tile([128, NB], dt.float32, tag="edt", name=f"edt{par}"))
            for chn in range(16):
                rowt = rows_p.tile([128, 256], dt.bfloat16, tag="rowt")
                nc.vector.memset(rowt[:, 66:128], 0.0)
                nc.vector.memset(rowt[:, 194:256], 0.0)
                for par in (0, 1):
                    base = par * 128
                    ps = ps_h.tile([128, 64], dt.float32, space="PSUM",
                                   tag="psh")
                    nc.tensor.matmul(ps[:], lhsT=lhsT_of(par, chn), rhs=Wmat[:],
                                     start=True, stop=True)
                    sc = scr64.tile([128, 64], dt.float32, tag="scr64")
                    esc = col_p.tile([128, 1], dt.float32, tag="esc")
                    nc.vector.tensor_mul(sc[:], ps[:], a_s[:])
                    nc.vector.tensor_reduce(esc[:], sc[:], axis=AX.X,
                                            op=Alu.add)
                    sc2 = scr64.tile([128, 64], dt.float32, tag="scr64")
                    nc.vector.tensor_mul(sc2[:], ps[:], a_d[:])
                    nc.vector.tensor_reduce(edts[par][:, chn:chn + 1], sc2[:],
                                            axis=AX.X, op=Alu.add)
                    nc.vector.tensor_copy(rowt[:, base:base + 64], ps[:])
                    nc.vector.tensor_copy(rowt[:, base + 64:base + 65], esc[:])
                    hi32 = col_p.tile([128, 1], dt.float32, tag="hi32")
                    nc.vector.tensor_copy(hi32[:], rowt[:, base + 64:base + 65])
                    nc.vector.tensor_sub(rowt[:, base + 65:base + 66],
                                         esc[:], hi32[:])
                nc.sync.dma_start(
                    out=htab[layer][pair, chn * 128:(chn + 1) * 128, :],
                    in_=rowt[:])
            nc.sync.dma_start(out=htab[layer][pair, DUM:DUM + 1, :],
                              in_=dum_d[:])
            return edts

        def sparse_phase(layer, pair, edts):
            """Per-block gather + GAT softmax/aggregate for both graphs."""
            sp_lvl = os.environ.get("K_SP", "full")
            bias_R = a_R["b1"] if layer == 1 else a_R["b2"]
            for b in range(NB):
                D = dmax[b]
                msg = msg_p.tile([128, D, 256], dt.bfloat16, tag="msg")
                nc.gpsimd.dma_gather(
                    msg[:], htab[layer][pair],
                    eidx_s[:, 8 * int(coloff[b]): 8 * int(coloff[b] + D)],
                    128 * D, 128 * D, 256, single_packet=False)
                if layer == 1:
                    outrow = rows_p.tile([128, 256], dt.bfloat16, tag="orow")
                    nc.vector.memset(outrow[:, 64:128], 0.0)
                    nc.vector.memset(outrow[:, 192:256], 0.0)
                for par in (0, 1):
                    base = par * 128
                    if sp_lvl == "g":
                        if layer == 1:
                            nc.vector.tensor_copy(outrow[:, base:base + 64],
                                                  msg[:, 0, base:base + 64])
                        continue
                    est = wk_p.tile([128, D], dt.float32, tag="est")
                    nc.vector.tensor_add(est[:], msg[:, :, base + 64],
                                         msg[:, :, base + 65])
                    nc.vector.tensor_scalar_add(est[:], est[:],
                                                edts[par][:, b:b + 1])
                    lr = wk_p.tile([128, D], dt.float32, tag="lrl")
                    nc.vector.tensor_scalar_mul(lr[:], est[:], 0.2)
                    nc.vector.tensor_tensor(out=est[:], in0=est[:], in1=lr[:],
                                            op=Alu.max)
                    if sp_lvl == "e":
                        if layer == 1:
                            nc.vector.tensor_copy(outrow[:, base:base + 64],
                                                  msg[:, 0, base:base + 64])
                        continue
                    mneg = col_p.tile([128, 1], dt.float32, tag="mneg")
                    nc.vector.tensor_reduce(mneg[:], est[:], axis=AX.X,
                                            op=Alu.max, negate=True)
                    wbf = wk_p.tile([128, D], dt.bfloat16, tag="wbf")
                    nc.scalar.activation(wbf[:], est[:], Act.Exp,
                                         bias=mneg[:], scale=1.0)
                    scol = col_p.tile([128, 1], dt.float32, tag="scol")
                    nc.vector.tensor_reduce(scol[:], wbf[:], axis=AX.X,
                                            op=Alu.add)
                    nc.vector.tensor_scalar_add(scol[:], scol[:], 1e-16)
                    rcol = col_p.tile([128, 1], dt.float32, tag="rcol")
                    nc.vector.reciprocal(rcol[:], scol[:])
                    wmT = wm_p.tile([128, D, 64], dt.bfloat16, tag="wmT")
                    nc.vector.tensor_tensor(
                        out=wmT[:], in0=msg[:, :, base:base + 64],
                        in1=blast(wbf[:], 64), op=Alu.mult)
                    if sp_lvl == "w":
                        if layer == 1:
                            nc.vector.tensor_copy(outrow[:, base:base + 64],
                                                  msg[:, 0, base:base + 64])
                        continue
                    agg = agg_p.tile([128, 64], dt.float32, tag="agg")
                    nc.vector.tensor_reduce(
                        agg[:], wmT[:].rearrange("p d h -> p h d"),
                        axis=AX.X, op=Alu.add)
                    agg2 = agg_p.tile([128, 64], dt.float32, tag="agg2")
                    nc.vector.tensor_scalar_mul(agg2[:], agg[:], rcol[:])
                    nc.vector.tensor_add(agg2[:], agg2[:], bias_R[:])
                    if layer == 1:
                        nc.scalar.activation(outrow[:, base:base + 64],
                                             agg2[:], Act.Gelu)
                    else:
                        lrow = rows_p.tile([128, 128], dt.bfloat16, tag="lrow")
                        nc.vector.memset(lrow[:, 0:64], 0.0)
                        nc.scalar.activation(lrow[:, 64:128], agg2[:],
                                             Act.Gelu)
                        lg = 2 * pair + par
                        nc.sync.dma_start(
                            out=lstmtab[lg * TROWS + b * 128:
                                        lg * TROWS + (b + 1) * 128, :],
                            in_=lrow[:])
                if layer == 1:
                    nc.sync.dma_start(
                        out=l1out[pair, b * 128:(b + 1) * 128, :],
                        in_=outrow[:])
            if layer == 1:
                nc.sync.dma_start(out=l1out[pair, DUM:DUM + 1, :],
                                  in_=dum_d[:])

        # ---- layer 1 ----
        n_l1_pairs = 1 if stop in ("tab", "gath1") else 3  # noqa
        for pair in range(n_l1_pairs):
            def l1_lhsT(par, chn, pair=pair):
                g = 2 * pair + par
                return hfcT[:, g * NPAD + chn * 128: g * NPAD + (chn + 1) * 128]
            edts = table_build(1, pair, l1_lhsT)
            if stop != "tab":
                sparse_phase(1, pair, edts)

        fc_ctx.close()

        # ---- allgather ----
        if stop not in ("l1", "tab", "gath1"):
            nc.gpsimd.collective_compute(
            "AllGather", Alu.bypass,
            replica_groups=[[0, 1, 2, 3], [4, 5, 6, 7]],
                ins=[l1out[:].opt()], outs=[agtab[:].opt()])

        # ---- layer 2 ----
        n_l2_pairs = 0 if stop in ("l1", "ag", "tab", "gath1") else 3
        l2_ctx = contextlib.ExitStack()
        if n_l2_pairs:
            x2_p = l2_ctx.enter_context(tc.tile_pool(name="x2", bufs=2))
            x2s_p = l2_ctx.enter_context(tc.tile_pool(name="x2s", bufs=2))
            mask_p = l2_ctx.enter_context(tc.tile_pool(name="mask", bufs=2))
        for pair in range(n_l2_pairs):
            x2sel = []
            for par in (0, 1):
                lg = 2 * pair + par
                xg = x2_p.tile([128, 2, NPAD], dt.bfloat16, tag="xg")
                nc.gpsimd.dma_gather(xg[:], agtab[:], x2idx_s[:, lg, :],
                                     NPAD, NPAD, 256, transpose=True,
                                     single_packet=False)
                msk = mask_p.tile([128, NPAD], dt.uint8, tag="msk")
                nc.gpsimd.dma_start(out=msk[0:64, :],
                                    in_=bcast_ap(pmask_d[lg:lg + 1, :], 64,
                                                 NPAD))
                xs = x2s_p.tile([64, NPAD], dt.bfloat16, tag="xs", name=f"xs{par}")
                nc.vector.tensor_copy(xs[:], xg[0:64, 0, :])
                nc.vector.copy_predicated(xs[:], msk[0:64, :],
                                          xg[0:64, 1, :])
                x2sel.append(xs)

            def l2_lhsT(par, chn, x2sel=x2sel):
                return x2sel[par][:, chn * 128:(chn + 1) * 128]
            edts = table_build(2, pair, l2_lhsT)
            sparse_phase(2, pair, edts)

        for lg in range(6 if n_l2_pairs == 3 else 0):
            nc.sync.dma_start(
                out=lstmtab[lg * TROWS + DUM: lg * TROWS + DUM + 1, :],
                in_=dum_d[0:1, 0:128])
        l2_ctx.close()

        # ---- LSTM ----
        # lstmtab rows pack gelu at lanes 64:128, so the transpose-gather
        # lands x at partitions 64:128; LSTM state (h/c/gates) lives at
        # partitions 0:64 so every matmul/psum AP starts at partition 0.
        if stop == "full":
            tail_lvl = os.environ.get("K_TAIL", "full")
            l2_ctx.close()
            lst_ctx = contextlib.ExitStack()
            big = lst_ctx.enter_context(tc.tile_pool(name="big", bufs=1))
            lst_p = lst_ctx.enter_context(tc.tile_pool(name="lst", bufs=2))
            XT = big.tile([128, T, 512], dt.bfloat16, name="XT")
            for tg in range(T):
                nc.gpsimd.dma_gather(XT[:, tg:tg + 1, :], lstmtab[:],
                                     lidx_s[:, tg, :], 512, 500, 128,
                                     transpose=True, single_packet=False)
            stacked = big.tile([128, SEQ], dt.bfloat16, name="stacked")
            cT = big.tile([64, SEQ], dt.float32, name="cT")
            h23 = big.tile([64, SEQ], dt.float32, name="h23")
            nc.vector.memset(stacked[0:64, :], 0.0)
            nc.vector.memset(cT[:], 0.0)
            for t in range(T if tail_lvl != "xt" else 0):
                nc.vector.tensor_copy(stacked[64:128, :],
                                      XT[64:128, t, 0:SEQ])
                zs = []
                for g in range(4):
                    ps = ps_z.tile([64, SEQ], dt.float32, space="PSUM",
                                   tag="pslstm", name=f"z{g}")
                    nc.tensor.matmul(ps[:], lhsT=LL_s[:, g * 64:(g + 1) * 64],
                                     rhs=stacked[:], start=True, stop=True)
                    zs.append(ps)
                gates = [None] * 4
                for g, fn in ((0, Act.Sigmoid), (1, Act.Sigmoid),
                              (3, Act.Sigmoid), (2, Act.Tanh)):
                    gt = lst_p.tile([64, SEQ], dt.float32, tag=f"gate{g}",
                                    name=f"gate{g}")
                    nc.scalar.activation(gt[:], zs[g][:], fn,
                                         bias=lstmb_s[:, g:g + 1], scale=1.0)
                    gates[g] = gt
                ig = lst_p.tile([64, SEQ], dt.float32, tag="ig")
                nc.vector.tensor_mul(ig[:], gates[0][:], gates[2][:])
                nc.vector.tensor_mul(cT[:], gates[1][:], cT[:])
                nc.vector.tensor_add(cT[:], cT[:], ig[:])
                th = lst_p.tile([64, SEQ], dt.float32, tag="th")
                nc.scalar.activation(th[:], cT[:], Act.Tanh)
                if t < T - 1:
                    nc.vector.tensor_mul(stacked[0:64, :], gates[3][:], th[:])
                else:
                    nc.vector.tensor_mul(h23[:], gates[3][:], th[:])

            # ---- LayerNorm (transposed; stats via ones-matmuls) ----
            if tail_lvl in ("xt", "lstm"):
                nc.vector.memset(h23[:], 0.5)
            if tail_lvl != "xt2":
                ps_mu = ps_misc.tile([1, SEQ], dt.float32, space="PSUM", tag="psmisc",
                                     name="ps_mu")
                nc.tensor.matmul(ps_mu[:], lhsT=onescol[0:64, :], rhs=h23[:],
                                 start=True, stop=True)
                mu_sb = lst_p.tile([1, SEQ], dt.float32, tag="mu")
                nc.scalar.activation(mu_sb[:], ps_mu[:], Act.Copy, scale=1.0 / 64)
                ps_mub = ps_misc.tile([64, SEQ], dt.float32, space="PSUM",
                                      tag="psb500", name="ps_mub")
                nc.tensor.matmul(ps_mub[:], lhsT=onesrow[:], rhs=mu_sb[:],
                                 start=True, stop=True)
                dtl = lst_p.tile([64, SEQ], dt.float32, tag="dtl")
                nc.vector.tensor_sub(dtl[:], h23[:], ps_mub[:])
                sq = lst_p.tile([64, SEQ], dt.float32, tag="sq")
                nc.vector.tensor_mul(sq[:], dtl[:], dtl[:])
                ps_var = ps_misc.tile([1, SEQ], dt.float32, space="PSUM", tag="psmisc",
                                      name="ps_var")
                nc.tensor.matmul(ps_var[:], lhsT=onescol[0:64, :], rhs=sq[:],
                                 start=True, stop=True)
                sd_sb = lst_p.tile([1, SEQ], dt.float32, tag="sd")
                nc.scalar.activation(sd_sb[:], ps_var[:], Act.Sqrt, bias=epsT[:],
                                     scale=1.0 / 64)
                rstd = lst_p.tile([1, SEQ], dt.float32, tag="rstd")
                nc.vector.reciprocal(rstd[:], sd_sb[:])
                ps_rb = ps_misc.tile([64, SEQ], dt.float32, space="PSUM",
                                     tag="psb500", name="ps_rb")
                nc.tensor.matmul(ps_rb[:], lhsT=onesrow[:], rhs=rstd[:],
                                 start=True, stop=True)
                hn = lst_p.tile([64, SEQ], dt.float32, tag="hn")
                nc.vector.tensor_mul(hn[:], dtl[:], ps_rb[:])
                nc.vector.tensor_scalar(out=hn[:], in0=hn[:],
                                        scalar1=lng_s[:], scalar2=lnb_s[:],
                                        op0=Alu.mult, op1=Alu.add)
                # ---- dense ----
                for q in range(4):
                    cs = min(128, SEQ - q * 128)
                    psd = ps_misc.tile([128, 12], dt.float32, space="PSUM",
                                       tag="psmisc", name="psd")
                    nc.tensor.matmul(psd[0:cs, :],
                                     lhsT=hn[:, q * 128:q * 128 + cs],
                                     rhs=dW_s[:], start=True, stop=True)
                    ot = lst_p.tile([128, 12], dt.float32, tag="ot")
                    nc.vector.tensor_add(ot[0:cs, :], psd[0:cs, :], dbR[0:cs, :])
                    nc.sync.dma_start(out=out_d[q * 128:q * 128 + cs, :],
                                      in_=ot[0:cs, :])
            lst_ctx.close()
        ctx.close()

    nc.compile()
    return nc


# ---------------------------------------------------------------------------
# Runner
# ---------------------------------------------------------------------------

_CACHE = {}


def _get_program_and_maps(inputs):
    st = prep_structure(inputs["edge_src"], inputs["edge_dst"])
    w = prep_weights(inputs)
    meta = dict(dmax=st["dmax"], coloff=st["coloff"], ncol=st["ncol"])
    key = ("prog", tuple(st["dmax"]), os.environ.get("K_STOP", "full"))
    if key not in _CACHE:
        _CACHE[key] = build_program(meta)
    nc = _CACHE[key]

    shared = dict(eidx=st["eidx"], lstmidx=st["lstmidx"], fcW=w["fcW"],
                  fcb=w["fcb"], W1=w["W1"], W2=w["W2"], a1s=w["a1s"],
                  a1d=w["a1d"], a2s=w["a2s"], a2d=w["a2d"], b1=w["b1"],
                  b2=w["b2"], LL=w["LL"], lstmb=w["lstmb"], lng=w["lng"],
                  lnb=w["lnb"], dW=w["dW"], db=w["db"], dumrow=w["dumrow"])
    in_maps = []
    for core in range(NCORES):
        pc = prep_core(st, core)
        m = dict(shared)
        m["xT"] = prep_xT(inputs, st, core)
        m["x2idx"] = pc["x2idx"]
        m["pmask"] = pc["pmask"]
        in_maps.append(m)
    return nc, in_maps


def run_on_hw(inputs, trace=False):
    _ensure_hook()
    from concourse.bass_utils import run_bass_kernel_spmd
    nc, in_maps = _get_program_and_maps(inputs)
    res = run_bass_kernel_spmd(nc, in_maps, list(range(NCORES)), trace=trace)
    out_full = np.zeros((B, N, PL, 1), np.float32)
    for core in range(NCORES):
        bb, c = core // 4, core % 4
        out_full[bb, 500 * c:500 * (c + 1), :, 0] = res.results[core]["out"]
    return out_full, res


def kernel(x, edge_src, edge_dst, fc_W, fc_b,
           g1_W, g1_b, g1_asrc, g1_adst,
           g2_W, g2_b, g2_asrc, g2_adst,
           lstm_Wih, lstm_Whh, lstm_bih, lstm_bhh,
           ln_g, ln_b, dense_W, dense_b):
    inputs = dict(x=x, edge_src=edge_src, edge_dst=edge_dst, fc_W=fc_W,
                  fc_b=fc_b, g1_W=g1_W, g1_b=g1_b, g1_asrc=g1_asrc,
                  g1_adst=g1_adst, g2_W=g2_W, g2_b=g2_b, g2_asrc=g2_asrc,
                  g2_adst=g2_adst, lstm_Wih=lstm_Wih, lstm_Whh=lstm_Whh,
                  lstm_bih=lstm_bih, lstm_bhh=lstm_bhh, ln_g=ln_g, ln_b=ln_b,
                  dense_W=dense_W, dense_b=dense_b)
    out, _ = run_on_hw(inputs, trace=False)
    return out



# revision 2
# speedup vs baseline: 335.9802x; 335.9802x over previous
"""GCN(GAT)x2 + LSTM + LN + dense on 8 Trainium2 NeuronCores (Bass/Tile).

Sharding: data-parallel over the B*T=48 graph replicas (6 per core, as 3
pair-tables); the layer1->layer2 scramble is an AllGather within each
4-core b-group; the LSTM stage consumes the core-local layer2 output
(the block assignment makes that transition local). All permutations /
sparse gathers are host-baked int16 index arrays driven through the
dma_gather (embedding-gather) DMA instruction against per-graph row
tables; the per-dst padded slot layout turns the segment softmax +
weighted aggregation into dense per-partition vector ops.
"""

import os
import sys
import numpy as np
import ml_dtypes

BF16 = ml_dtypes.bfloat16

N = 2000
NPAD = 2048
T = 24
B = 2
NX = 8
H = 64
E = 32000
PL = 12
NB = 16          # dst blocks of 128 pi-positions
DUM = 2048       # dummy table row index
TROWS = 2049     # rows per graph in a pair table
NEG = -1e9
LN_EPS = 1e-5
NCORES = 8
SEQ = 500        # LSTM sequences per core


def _ensure_hook():
    """Register the NTFF profile hook if the boot didn't (enables traces)."""
    try:
        try:
            import antenv.axon_hooks  # noqa: F401
        except ImportError:
            # The image's antenv lacks axon_hooks — synthesize it so
            # bass_utils' trace path finds the hook registry.
            import types
            import antenv
            mod = types.ModuleType("antenv.axon_hooks")
            mod._hook = None

            def set_axon_ntff_profile_hook(h, _m=mod):
                _m._hook = h

            def get_axon_ntff_profile_hook(_m=mod):
                return _m._hook

            mod.set_axon_ntff_profile_hook = set_axon_ntff_profile_hook
            mod.get_axon_ntff_profile_hook = get_axon_ntff_profile_hook
            sys.modules["antenv.axon_hooks"] = mod
            antenv.axon_hooks = mod
        from antenv.axon_hooks import (get_axon_ntff_profile_hook,
                                       set_axon_ntff_profile_hook)
        if get_axon_ntff_profile_hook() is None:
            from trn_agent_boot.trn_boot import _ntff_profile_via_ctypes
            so = "/opt/axon/libaxon_pjrt.so"
            if os.path.exists(so):
                set_axon_ntff_profile_hook(_ntff_profile_via_ctypes(so))
    except Exception:
        pass


# ---------------------------------------------------------------------------
# Host-side prep
# ---------------------------------------------------------------------------

def fmt_idx(flat):
    flat = np.asarray(flat)
    assert len(flat) % 128 == 0
    fmt = flat.reshape(-1, 16).T.astype(np.int16)
    return np.tile(fmt, (8, 1))


def prep_structure(edge_src, edge_dst):
    src = np.asarray(edge_src, np.int64)
    dst = np.asarray(edge_dst, np.int64)
    deg = np.bincount(dst, minlength=N)
    pi = np.argsort(deg, kind="stable")
    pi_inv = np.empty(N, np.int64)
    pi_inv[pi] = np.arange(N)

    dpos = pi_inv[dst]
    order = np.argsort(dpos, kind="stable")
    src_s = src[order]
    dpos_s = dpos[order]
    cnt = np.bincount(dpos_s, minlength=NPAD)
    dmax = [int(max(1, cnt[b * 128:(b + 1) * 128].max())) for b in range(NB)]
    coloff = np.concatenate([[0], np.cumsum(dmax)]).astype(np.int64)
    ncol = int(coloff[-1])

    starts = np.concatenate([[0], np.cumsum(cnt)]).astype(np.int64)
    slot = np.full((ncol, 128), DUM, np.int64)
    b_of = dpos_s // 128
    p_of = dpos_s % 128
    jw = np.arange(E) - starts[dpos_s]
    slot[coloff[b_of] + jw, p_of] = pi_inv[src_s]
    eidx = fmt_idx(slot.reshape(-1))

    per_t = []
    for t in range(T):
        nl = np.arange(SEQ)
        flat_i = nl * T + t
        lrow = (flat_i // N) * TROWS + pi_inv[flat_i % N]
        lrow = np.concatenate([lrow, np.full(12, -1, np.int64)])
        per_t.append(fmt_idx(lrow))  # [128, 32]
    lstmidx = np.ascontiguousarray(np.stack(per_t, axis=1))  # [128, 24, 32]

    return dict(pi=pi, pi_inv=pi_inv, dmax=dmax, coloff=coloff, ncol=ncol,
                eidx=eidx, lstmidx=lstmidx)


def prep_core(st, core):
    pi, pi_inv = st["pi"], st["pi_inv"]
    c = core % 4
    x2rows = np.full((6, NPAD), DUM, np.int64)
    pmask = np.zeros((6, NPAD), np.float32)
    ppos = np.arange(N)
    for lg in range(6):
        t2 = 6 * c + lg
        flat_i = pi[ppos] * T + t2
        tp, npr = flat_i // N, flat_i % N
        x2rows[lg, :N] = (tp // 6 * 3 + (tp % 6) // 2) * TROWS + pi_inv[npr]
        pmask[lg, :N] = ((tp % 6) % 2).astype(np.float32)
    x2idx = np.stack([fmt_idx(x2rows[lg]) for lg in range(6)], axis=1)
    return dict(x2idx=np.ascontiguousarray(x2idx.astype(np.int16)),
                pmask=pmask.astype(np.uint8))


def prep_weights(p):
    f32 = np.float32
    w = {}
    w["fcW"] = np.asarray(p["fc_W"], f32)
    w["fcb"] = np.asarray(p["fc_b"], f32).reshape(64, 1)
    w["W1"] = np.asarray(p["g1_W"], f32)
    w["W2"] = np.asarray(p["g2_W"], f32).astype(BF16)
    w["a1s"] = np.asarray(p["g1_asrc"], f32).reshape(1, 64)
    w["a1d"] = np.asarray(p["g1_adst"], f32).reshape(1, 64)
    w["a2s"] = np.asarray(p["g2_asrc"], f32).reshape(1, 64)
    w["a2d"] = np.asarray(p["g2_adst"], f32).reshape(1, 64)
    w["b1"] = np.asarray(p["g1_b"], f32).reshape(1, 64)
    w["b2"] = np.asarray(p["g2_b"], f32).reshape(1, 64)
    Wih = np.asarray(p["lstm_Wih"], f32)
    Whh = np.asarray(p["lstm_Whh"], f32)
    w["LL"] = np.vstack([Whh.T, Wih.T]).astype(BF16)  # K rows: 0:64=h, 64:128=x
    bihh = np.asarray(p["lstm_bih"], f32) + np.asarray(p["lstm_bhh"], f32)
    w["lstmb"] = np.ascontiguousarray(bihh.reshape(4, 64).T)  # [64,4] i,f,g,o
    w["lng"] = np.asarray(p["ln_g"], f32).reshape(64, 1)
    w["lnb"] = np.asarray(p["ln_b"], f32).reshape(64, 1)
    w["dW"] = np.asarray(p["dense_W"], f32)
    w["db"] = np.asarray(p["dense_b"], f32).reshape(1, 12)
    dum = np.zeros((1, 256), f32)
    dum[0, 64] = NEG
    dum[0, 192] = NEG
    w["dumrow"] = dum.astype(BF16)
    return w


def prep_xT(p, st, core):
    x = np.asarray(p["x"], np.float32)
    b, c = core // 4, core % 4
    pi = st["pi"]
    xT = np.zeros((NX, 6 * NPAD), np.float32)
    for lg in range(6):
        xT[:, lg * NPAD: lg * NPAD + N] = x[b, pi, 6 * c + lg, :].T
    return np.ascontiguousarray(xT)


# ---------------------------------------------------------------------------
# Bass program
# ---------------------------------------------------------------------------

def build_program(meta, stop=None):
    stop = stop or os.environ.get("K_STOP", "full")
    import concourse.bass as bass
    import concourse.mybir as mybir
    import concourse.tile as tile
    from concourse import bacc

    dt = mybir.dt
    Alu = mybir.AluOpType
    Act = mybir.ActivationFunctionType
    AX = mybir.AxisListType
    dmax, coloff, ncol = meta["dmax"], meta["coloff"], meta["ncol"]

    nc = bacc.Bacc("TRN2", target_bir_lowering=False, debug=False,
                   num_devices=NCORES)

    def din(name, shape, dty):
        return nc.dram_tensor(name, list(shape), dty, kind="ExternalInput")

    xT_d = din("xT", (NX, 6 * NPAD), dt.float32)
    eidx_d = din("eidx", (128, ncol * 8), dt.int16)
    x2idx_d = din("x2idx", (128, 6, 128), dt.int16)
    lstmidx_d = din("lstmidx", (128, T, 32), dt.int16)
    pmask_d = din("pmask", (6, NPAD), dt.uint8)
    fcW_d = din("fcW", (NX, 64), dt.float32)
    fcb_d = din("fcb", (64, 1), dt.float32)
    W1_d = din("W1", (64, 64), dt.float32)
    W2_d = din("W2", (64, 64), dt.bfloat16)
    a1s_d = din("a1s", (1, 64), dt.float32)
    a1d_d = din("a1d", (1, 64), dt.float32)
    a2s_d = din("a2s", (1, 64), dt.float32)
    a2d_d = din("a2d", (1, 64), dt.float32)
    b1_d = din("b1", (1, 64), dt.float32)
    b2_d = din("b2", (1, 64), dt.float32)
    LL_d = din("LL", (128, 256), dt.bfloat16)
    lstmb_d = din("lstmb", (64, 4), dt.float32)
    lng_d = din("lng", (64, 1), dt.float32)
    lnb_d = din("lnb", (64, 1), dt.float32)
    dW_d = din("dW", (64, 12), dt.float32)
    db_d = din("db", (1, 12), dt.float32)
    dum_d = din("dumrow", (1, 256), dt.bfloat16)
    out_d = nc.dram_tensor("out", [SEQ, 12], dt.float32, kind="ExternalOutput")

    def bcast_ap(dram_t, parts, free):
        """Partition-broadcast read AP of a [1, free] DRAM tensor."""
        ap = dram_t if isinstance(dram_t, bass.AP) else dram_t.ap()
        return bass.AP(tensor=ap.tensor, offset=ap.offset,
                       ap=[[0, parts]] + [list(x) for x in ap.ap[1:]])

    def blast(ap_, n):
        """Append a step-0 dim of size n to an AP (free-dim broadcast)."""
        return bass.AP(tensor=ap_.tensor, offset=ap_.offset,
                       ap=list(ap_.ap) + [[0, n]])

    with tile.TileContext(nc) as tc:
        import contextlib
        ctx = contextlib.ExitStack()
        consts = ctx.enter_context(tc.tile_pool(name="consts", bufs=1))
        dramp = ctx.enter_context(tc.tile_pool(name="dramp", bufs=1,
                                               space="DRAM"))
        ps_h = ctx.enter_context(tc.tile_pool(name="ps_h", bufs=2,
                                              space="PSUM"))
        ps_misc = ctx.enter_context(tc.tile_pool(name="ps_misc", bufs=1,
                                                 space="PSUM"))
        ps_z = ctx.enter_context(tc.tile_pool(name="ps_z", bufs=4,
                                              space="PSUM"))
        rows_p = ctx.enter_context(tc.tile_pool(name="rows", bufs=3))
        msg_p = ctx.enter_context(tc.tile_pool(name="msg", bufs=2))
        wk_p = ctx.enter_context(tc.tile_pool(name="wk", bufs=6))
        col_p = ctx.enter_context(tc.tile_pool(name="col", bufs=8))
        edt_p = ctx.enter_context(tc.tile_pool(name="edt", bufs=4))
        wm_p = ctx.enter_context(tc.tile_pool(name="wm", bufs=3))
        agg_p = ctx.enter_context(tc.tile_pool(name="agg", bufs=6))

        # ---- constants into SBUF ----
        def load(name, shape, dty, src_ap):
            t = consts.tile(shape, dty, name=name)
            nc.sync.dma_start(out=t[:], in_=src_ap)
            return t

        eidx_s = load("eidx_s", [128, ncol * 8], dt.int16, eidx_d[:])
        x2idx_s = load("x2idx_s", [128, 6, 128], dt.int16, x2idx_d[:])
        lidx_s = load("lidx_s", [128, T, 32], dt.int16, lstmidx_d[:])
        fcW_s = load("fcW_s", [NX, 64], dt.float32, fcW_d[:])
        fcb_s = load("fcb_s", [64, 1], dt.float32, fcb_d[:])
        W1_s = load("W1_s", [64, 64], dt.float32, W1_d[:])
        W2_s = load("W2_s", [64, 64], dt.bfloat16, W2_d[:])
        LL_s = load("LL_s", [128, 256], dt.bfloat16, LL_d[:])
        dW_s = load("dW_s", [64, 12], dt.float32, dW_d[:])
        lstmb_s = load("lstmb_s", [64, 4], dt.float32, lstmb_d[:])
        lng_s = load("lng_s", [64, 1], dt.float32, lng_d[:])
        lnb_s = load("lnb_s", [64, 1], dt.float32, lnb_d[:])
        # partition-broadcast constants
        a_R = {}
        for nm, d_ in (("a1s", a1s_d), ("a1d", a1d_d), ("a2s", a2s_d),
                       ("a2d", a2d_d), ("b1", b1_d), ("b2", b2_d)):
            t = consts.tile([128, 64], dt.float32, name=nm + "R")
            nc.gpsimd.dma_start(out=t[:], in_=bcast_ap(d_, 128, 64))
            a_R[nm] = t
        dbR = consts.tile([128, 12], dt.float32, name="dbR")
        nc.gpsimd.dma_start(out=dbR[:], in_=bcast_ap(db_d, 128, 12))
        onesrow = consts.tile([1, 64], dt.float32, name="onesrow")
        nc.vector.memset(onesrow[:], 1.0)
        onescol = consts.tile([128, 1], dt.float32, name="onescol")
        nc.vector.memset(onescol[:], 1.0)
        epsT = consts.tile([1, 1], dt.float32, name="epsT")
        nc.vector.memset(epsT[:], LN_EPS)

        # ---- DRAM tables ----
        htab = {1: dramp.tile([3, TROWS, 256], dt.bfloat16, name="htab1"),
                2: dramp.tile([3, TROWS, 256], dt.bfloat16, name="htab2")}
        l1out = dramp.tile([3, TROWS, 256], dt.bfloat16, name="l1out")
        agtab = dramp.tile([12 * TROWS, 256], dt.bfloat16, name="agtab")
        lstmtab = dramp.tile([6 * TROWS, 128], dt.bfloat16, name="lstmtab")

        # ---- FC: hfcT [64, 12288] = fcW.T @ xT + fcb ----
        fc_ctx = contextlib.ExitStack()
        hfc_p = fc_ctx.enter_context(tc.tile_pool(name="hfc", bufs=1))
        hfcT = hfc_p.tile([64, 6 * NPAD], dt.float32, name="hfcT")
        with tc.tile_pool(name="xtp", bufs=1) as xt_p:
            xT_s = xt_p.tile([NX, 6 * NPAD], dt.float32, name="xT_s")
            nc.sync.dma_start(out=xT_s[:], in_=xT_d[:])
            for chix in range(6 * NPAD // 512):
                sl = slice(chix * 512, (chix + 1) * 512)
                ps = ps_z.tile([64, 512], dt.float32, space="PSUM",
                               tag="pslstm", name="psfc")
                nc.tensor.matmul(ps[:], lhsT=fcW_s[:], rhs=xT_s[:, sl],
                                 start=True, stop=True)
                nc.vector.tensor_scalar_add(hfcT[:, sl], ps[:], fcb_s[:])

        scr64 = wk_p  # alias for readability

        def table_build(layer, pair, lhsT_of):
            """Build pair h-table (rows [128,256]) + return ed tiles."""
            Wmat = W1_s if layer == 1 else W2_s
            a_s = a_R["a1s"] if layer == 1 else a_R["a2s"]
            a_d = a_R["a1d"] if layer == 1 else a_R["a2d"]
            edts = []
            for par in (0, 1):
                edts.append(edt_p.tile([128, NB], dt.float32, tag="edt", name=f"edt{par}"))
            for chn in range(16):
                rowt = rows_p.tile([128, 256], dt.bfloat16, tag="rowt")
                nc.vector.memset(rowt[:, 66:128], 0.0)
                nc.vector.memset(rowt[:, 194:256], 0.0)
                for par in (0, 1):
                    base = par * 128
                    ps = ps_h.tile([128, 64], dt.float32, space="PSUM",
                                   tag="psh")
                    nc.tensor.matmul(ps[:], lhsT=lhsT_of(par, chn), rhs=Wmat[:],
                                     start=True, stop=True)
                    sc = scr64.tile([128, 64], dt.float32, tag="scr64")
                    esc = col_p.tile([128, 1], dt.float32, tag="esc")
                    nc.vector.tensor_mul(sc[:], ps[:], a_s[:])
                    nc.vector.tensor_reduce(esc[:], sc[:], axis=AX.X,
                                            op=Alu.add)
                    sc2 = scr64.tile([128, 64], dt.float32, tag="scr64")
                    nc.vector.tensor_mul(sc2[:], ps[:], a_d[:])
                    nc.vector.tensor_reduce(edts[par][:, chn:chn + 1], sc2[:],
                                            axis=AX.X, op=Alu.add)
                    nc.vector.tensor_copy(rowt[:, base:base + 64], ps[:])
                    nc.vector.tensor_copy(rowt[:, base + 64:base + 65], esc[:])
                    hi32 = col_p.tile([128, 1], dt.float32, tag="hi32")
                    nc.vector.tensor_copy(hi32[:], rowt[:, base + 64:base + 65])
                    nc.vector.tensor_sub(rowt[:, base + 65:base + 66],
                                         esc[:], hi32[:])
                nc.sync.dma_start(
                    out=htab[layer][pair, chn * 128:(chn + 1) * 128, :],
                    in_=rowt[:])
            nc.sync.dma_start(out=htab[layer][pair, DUM:DUM + 1, :],
                              in_=dum_d[:])
            return edts

        def sparse_phase(layer, pair, edts):
            """Per-block gather + GAT softmax/aggregate for both graphs."""
            sp_lvl = os.environ.get("K_SP", "full")
            bias_R = a_R["b1"] if layer == 1 else a_R["b2"]
            for b in range(NB):
                D = dmax[b]
                msg = msg_p.tile([128, D, 256], dt.bfloat16, tag="msg")
                nc.gpsimd.dma_gather(
                    msg[:], htab[layer][pair],
                    eidx_s[:, 8 * int(coloff[b]): 8 * int(coloff[b] + D)],
                    128 * D, 128 * D, 256, single_packet=False)
                if layer == 1:
                    outrow = rows_p.tile([128, 256], dt.bfloat16, tag="orow")
                    nc.vector.memset(outrow[:, 64:128], 0.0)
                    nc.vector.memset(outrow[:, 192:256], 0.0)
                for par in (0, 1):
                    base = par * 128
                    if sp_lvl == "g":
                        if layer == 1:
                            nc.vector.tensor_copy(outrow[:, base:base + 64],
                                                  msg[:, 0, base:base + 64])
                        continue
                    est = wk_p.tile([128, D], dt.float32, tag="est")
                    nc.vector.tensor_add(est[:], msg[:, :, base + 64],
                                         msg[:, :, base + 65])
                    nc.vector.tensor_scalar_add(est[:], est[:],
                                                edts[par][:, b:b + 1])
                    lr = wk_p.tile([128, D], dt.float32, tag="lrl")
                    nc.vector.tensor_scalar_mul(lr[:], est[:], 0.2)
                    nc.vector.tensor_tensor(out=est[:], in0=est[:], in1=lr[:],
                                            op=Alu.max)
                    if sp_lvl == "e":
                        if layer == 1:
                            nc.vector.tensor_copy(outrow[:, base:base + 64],
                                                  msg[:, 0, base:base + 64])
                        continue
                    mneg = col_p.tile([128, 1], dt.float32, tag="mneg")
                    nc.vector.tensor_reduce(mneg[:], est[:], axis=AX.X,
                                            op=Alu.max, negate=True)
                    wbf = wk_p.tile([128, D], dt.bfloat16, tag="wbf")
                    nc.scalar.activation(wbf[:], est[:], Act.Exp,
                                         bias=mneg[:], scale=1.0)
                    scol = col_p.tile([128, 1], dt.float32, tag="scol")
                    nc.vector.tensor_reduce(scol[:], wbf[:], axis=AX.X,
                                            op=Alu.add)
                    nc.vector.tensor_scalar_add(scol[:], scol[:], 1e-16)
                    rcol = col_p.tile([128, 1], dt.float32, tag="rcol")
                    nc.vector.reciprocal(rcol[:], scol[:])
                    wmT = wm_p.tile([128, D, 64], dt.bfloat16, tag="wmT")
                    nc.vector.tensor_tensor(
                        out=wmT[:], in0=msg[:, :, base:base + 64],
                        in1=blast(wbf[:], 64), op=Alu.mult)
                    if sp_lvl == "w":
                        if layer == 1:
                            nc.vector.tensor_copy(outrow[:, base:base + 64],
                                                  msg[:, 0, base:base + 64])
                        continue
                    agg = agg_p.tile([128, 64], dt.float32, tag="agg")
                    nc.vector.tensor_reduce(
                        agg[:], wmT[:].rearrange("p d h -> p h d"),
                        axis=AX.X, op=Alu.add)
                    agg2 = agg_p.tile([128, 64], dt.float32, tag="agg2")
                    nc.vector.tensor_scalar_mul(agg2[:], agg[:], rcol[:])
                    nc.vector.tensor_add(agg2[:], agg2[:], bias_R[:])
                    if layer == 1:
                        nc.scalar.activation(outrow[:, base:base + 64],
                                             agg2[:], Act.Gelu)
                    else:
                        lrow = rows_p.tile([128, 128], dt.bfloat16, tag="lrow")
                        nc.vector.memset(lrow[:, 0:64], 0.0)
                        nc.scalar.activation(lrow[:, 64:128], agg2[:],
                                             Act.Gelu)
                        lg = 2 * pair + par
                        nc.sync.dma_start(
                            out=lstmtab[lg * TROWS + b * 128:
                                        lg * TROWS + (b + 1) * 128, :],
                            in_=lrow[:])
                if layer == 1:
                    nc.sync.dma_start(
                        out=l1out[pair, b * 128:(b + 1) * 128, :],
                        in_=outrow[:])
            if layer == 1:
                nc.sync.dma_start(out=l1out[pair, DUM:DUM + 1, :],
                                  in_=dum_d[:])

        # ---- layer 1 ----
        n_l1_pairs = 1 if stop in ("tab", "gath1") else 3  # noqa
        for pair in range(n_l1_pairs):
            def l1_lhsT(par, chn, pair=pair):
                g = 2 * pair + par
                return hfcT[:, g * NPAD + chn * 128: g * NPAD + (chn + 1) * 128]
            edts = table_build(1, pair, l1_lhsT)
            if stop != "tab":
                sparse_phase(1, pair, edts)

        fc_ctx.close()

        # ---- allgather ----
        if stop not in ("l1", "tab", "gath1"):
            nc.gpsimd.collective_compute(
            "AllGather", Alu.bypass,
            replica_groups=[[0, 1, 2, 3], [4, 5, 6, 7]],
                ins=[l1out[:].opt()], outs=[agtab[:].opt()])

        # ---- layer 2 ----
        n_l2_pairs = 0 if stop in ("l1", "ag", "tab", "gath1") else 3
        l2_ctx = contextlib.ExitStack()
        if n_l2_pairs:
            x2_p = l2_ctx.enter_context(tc.tile_pool(name="x2", bufs=2))
            x2s_p = l2_ctx.enter_context(tc.tile_pool(name="x2s", bufs=2))
            mask_p = l2_ctx.enter_context(tc.tile_pool(name="mask", bufs=2))
        for pair in range(n_l2_pairs):
            x2sel = []
            for par in (0, 1):
                lg = 2 * pair + par
                xg = x2_p.tile([128, 2, NPAD], dt.bfloat16, tag="xg")
                nc.gpsimd.dma_gather(xg[:], agtab[:], x2idx_s[:, lg, :],
                                     NPAD, NPAD, 256, transpose=True,
                                     single_packet=False)
                msk = mask_p.tile([128, NPAD], dt.uint8, tag="msk")
                nc.gpsimd.dma_start(out=msk[0:64, :],
                                    in_=bcast_ap(pmask_d[lg:lg + 1, :], 64,
                                                 NPAD))
                xs = x2s_p.tile([64, NPAD], dt.bfloat16, tag="xs", name=f"xs{par}")
                nc.vector.tensor_copy(xs[:], xg[0:64, 0, :])
                nc.vector.copy_predicated(xs[:], msk[0:64, :],
                                          xg[0:64, 1, :])
                x2sel.append(xs)

            def l2_lhsT(par, chn, x2sel=x2sel):
                return x2sel[par][:, chn * 128:(chn + 1) * 128]
            edts = table_build(2, pair, l2_lhsT)
            sparse_phase(2, pair, edts)

        for lg in range(6 if n_l2_pairs == 3 else 0):
            nc.sync.dma_start(
                out=lstmtab[lg * TROWS + DUM: lg * TROWS + DUM + 1, :],
                in_=dum_d[0:1, 0:128])
        l2_ctx.close()

        # ---- LSTM ----
        # lstmtab rows pack gelu at lanes 64:128, so the transpose-gather
        # lands x at partitions 64:128; LSTM state (h/c/gates) lives at
        # partitions 0:64 so every matmul/psum AP starts at partition 0.
        if stop == "full":
            tail_lvl = os.environ.get("K_TAIL", "full")
            l2_ctx.close()
            lst_ctx = contextlib.ExitStack()
            big = lst_ctx.enter_context(tc.tile_pool(name="big", bufs=1))
            lst_p = lst_ctx.enter_context(tc.tile_pool(name="lst", bufs=2))
            XT = big.tile([128, T, 512], dt.bfloat16, name="XT")
            for tg in range(T):
                nc.gpsimd.dma_gather(XT[:, tg:tg + 1, :], lstmtab[:],
                                     lidx_s[:, tg, :], 512, 500, 128,
                                     transpose=True, single_packet=False)
            stacked = big.tile([128, SEQ], dt.bfloat16, name="stacked")
            cT = big.tile([64, SEQ], dt.float32, name="cT")
            h23 = big.tile([64, SEQ], dt.float32, name="h23")
            nc.vector.memset(stacked[0:64, :], 0.0)
            nc.vector.memset(cT[:], 0.0)
            for t in range(T if tail_lvl != "xt" else 0):
                nc.vector.tensor_copy(stacked[64:128, :],
                                      XT[64:128, t, 0:SEQ])
                zs = []
                for g in range(4):
                    ps = ps_z.tile([64, SEQ], dt.float32, space="PSUM",
                                   tag="pslstm", name=f"z{g}")
                    nc.tensor.matmul(ps[:], lhsT=LL_s[:, g * 64:(g + 1) * 64],
                                     rhs=stacked[:], start=True, stop=True)
                    zs.append(ps)
                gates = [None] * 4
                for g, fn in ((0, Act.Sigmoid), (1, Act.Sigmoid),
                              (3, Act.Sigmoid), (2, Act.Tanh)):
                    gt = lst_p.tile([64, SEQ], dt.float32, tag=f"gate{g}",
                                    name=f"gate{g}")
                    nc.scalar.activation(gt[:], zs[g][:], fn,
                                         bias=lstmb_s[:, g:g + 1], scale=1.0)
                    gates[g] = gt
                ig = lst_p.tile([64, SEQ], dt.float32, tag="ig")
                nc.vector.tensor_mul(ig[:], gates[0][:], gates[2][:])
                nc.vector.tensor_mul(cT[:], gates[1][:], cT[:])
                nc.vector.tensor_add(cT[:], cT[:], ig[:])
                th = lst_p.tile([64, SEQ], dt.float32, tag="th")
                nc.scalar.activation(th[:], cT[:], Act.Tanh)
                if t < T - 1:
                    nc.vector.tensor_mul(stacked[0:64, :], gates[3][:], th[:])
                else:
                    nc.vector.tensor_mul(h23[:], gates[3][:], th[:])

            # ---- LayerNorm (transposed; stats via ones-matmuls) ----
            if tail_lvl in ("xt", "lstm"):
                nc.vector.memset(h23[:], 0.5)
            if tail_lvl != "xt2":
                ps_mu = ps_misc.tile([1, SEQ], dt.float32, space="PSUM", tag="psmisc",
                                     name="ps_mu")
                nc.tensor.matmul(ps_mu[:], lhsT=onescol[0:64, :], rhs=h23[:],
                                 start=True, stop=True)
                mu_sb = lst_p.tile([1, SEQ], dt.float32, tag="mu")
                nc.scalar.activation(mu_sb[:], ps_mu[:], Act.Copy, scale=1.0 / 64)
                ps_mub = ps_misc.tile([64, SEQ], dt.float32, space="PSUM",
                                      tag="psb500", name="ps_mub")
                nc.tensor.matmul(ps_mub[:], lhsT=onesrow[:], rhs=mu_sb[:],
                                 start=True, stop=True)
                dtl = lst_p.tile([64, SEQ], dt.float32, tag="dtl")
                nc.vector.tensor_sub(dtl[:], h23[:], ps_mub[:])
                sq = lst_p.tile([64, SEQ], dt.float32, tag="sq")
                nc.vector.tensor_mul(sq[:], dtl[:], dtl[:])
                ps_var = ps_misc.tile([1, SEQ], dt.float32, space="PSUM", tag="psmisc",
                                      name="ps_var")
                nc.tensor.matmul(ps_var[:], lhsT=onescol[0:64, :], rhs=sq[:],
                                 start=True, stop=True)
                sd_sb = lst_p.tile([1, SEQ], dt.float32, tag="sd")
                nc.scalar.activation(sd_sb[:], ps_var[:], Act.Sqrt, bias=epsT[:],
                                     scale=1.0 / 64)
                rstd = lst_p.tile([1, SEQ], dt.float32, tag="rstd")
                nc.vector.reciprocal(rstd[:], sd_sb[:])
                ps_rb = ps_misc.tile([64, SEQ], dt.float32, space="PSUM",
                                     tag="psb500", name="ps_rb")
                nc.tensor.matmul(ps_rb[:], lhsT=onesrow[:], rhs=rstd[:],
                                 start=True, stop=True)
                hn = lst_p.tile([64, SEQ], dt.float32, tag="hn")
                nc.vector.tensor_mul(hn[:], dtl[:], ps_rb[:])
                nc.vector.tensor_scalar(out=hn[:], in0=hn[:],
                                        scalar1=lng_s[:], scalar2=lnb_s[:],
                                        op0=Alu.mult, op1=Alu.add)
                # ---- dense ----
                for q in range(4):
                    cs = min(128, SEQ - q * 128)
                    psd = ps_misc.tile([128, 12], dt.float32, space="PSUM",
                                       tag="psmisc", name="psd")
                    nc.tensor.matmul(psd[0:cs, :],
                                     lhsT=hn[:, q * 128:q * 128 + cs],
                                     rhs=dW_s[:], start=True, stop=True)
                    ot = lst_p.tile([128, 12], dt.float32, tag="ot")
                    nc.vector.tensor_add(ot[0:cs, :], psd[0:cs, :], dbR[0:cs, :])
                    nc.sync.dma_start(out=out_d[q * 128:q * 128 + cs, :],
                                      in_=ot[0:cs, :])
            lst_ctx.close()
        ctx.close()

    nc.compile()
    return nc


# ---------------------------------------------------------------------------
# Runner
# ---------------------------------------------------------------------------

_CACHE = {}


def _get_program_and_maps(inputs):
    st = prep_structure(inputs["edge_src"], inputs["edge_dst"])
    w = prep_weights(inputs)
    meta = dict(dmax=st["dmax"], coloff=st["coloff"], ncol=st["ncol"])
    key = ("prog", tuple(st["dmax"]), os.environ.get("K_STOP", "full"))
    if key not in _CACHE:
        _CACHE[key] = build_program(meta)
    nc = _CACHE[key]

    shared = dict(eidx=st["eidx"], lstmidx=st["lstmidx"], fcW=w["fcW"],
                  fcb=w["fcb"], W1=w["W1"], W2=w["W2"], a1s=w["a1s"],
                  a1d=w["a1d"], a2s=w["a2s"], a2d=w["a2d"], b1=w["b1"],
                  b2=w["b2"], LL=w["LL"], lstmb=w["lstmb"], lng=w["lng"],
                  lnb=w["lnb"], dW=w["dW"], db=w["db"], dumrow=w["dumrow"])
    in_maps = []
    for core in range(NCORES):
        pc = prep_core(st, core)
        m = dict(shared)
        m["xT"] = prep_xT(inputs, st, core)
        m["x2idx"] = pc["x2idx"]
        m["pmask"] = pc["pmask"]
        in_maps.append(m)
    return nc, in_maps


def run_on_hw(inputs, trace=False):
    _ensure_hook()
    from concourse.bass_utils import run_bass_kernel_spmd
    nc, in_maps = _get_program_and_maps(inputs)
    res = run_bass_kernel_spmd(nc, in_maps, list(range(NCORES)), trace=trace)
    out_full = np.zeros((B, N, PL, 1), np.float32)
    for core in range(NCORES):
        bb, c = core // 4, core % 4
        out_full[bb, 500 * c:500 * (c + 1), :, 0] = res.results[core]["out"]
    return out_full, res


def kernel(x, edge_src, edge_dst, fc_W, fc_b,
           g1_W, g1_b, g1_asrc, g1_adst,
           g2_W, g2_b, g2_asrc, g2_adst,
           lstm_Wih, lstm_Whh, lstm_bih, lstm_bhh,
           ln_g, ln_b, dense_W, dense_b):
    inputs = dict(x=x, edge_src=edge_src, edge_dst=edge_dst, fc_W=fc_W,
                  fc_b=fc_b, g1_W=g1_W, g1_b=g1_b, g1_asrc=g1_asrc,
                  g1_adst=g1_adst, g2_W=g2_W, g2_b=g2_b, g2_asrc=g2_asrc,
                  g2_adst=g2_adst, lstm_Wih=lstm_Wih, lstm_Whh=lstm_Whh,
                  lstm_bih=lstm_bih, lstm_bhh=lstm_bhh, ln_g=ln_g, ln_b=ln_b,
                  dense_W=dense_W, dense_b=dense_b)
    out, _ = run_on_hw(inputs, trace=False)
    return out



# revision 12
# speedup vs baseline: 745.6741x; 2.2194x over previous
"""GCN(GAT)x2 + LSTM + LN + dense on 8 Trainium2 NeuronCores (Bass/Tile).

V2: data-parallel over B*T=48 graphs (6 per core). All 6 graphs of a core
pack into ONE h-table (1KB rows: 6x64 h lanes + 6 es lanes), so the
per-edge slot gather costs one Q7 descriptor stream per layer instead of
three. es/ed come from the same matmul as h (rhs = [W | W@a_s | W@a_d]).
Softmax drops the max-shift (est bounded ~|6.5|, exp safe in f32). The
per-edge weighted slot-sum runs on the Tensor engine as identity-matmul
PSUM accumulation; DVE only does the est chain + one blast-multiply per
graph. LSTM stage unchanged (core-local via the scramble block structure).
"""

import os
import sys
import numpy as np
import ml_dtypes

BF16 = ml_dtypes.bfloat16

N = 2000
NPAD = 2048
T = 24
B = 2
NX = 8
H = 64
E = 32000
PL = 12
NB = 16          # dst blocks of 128 pi-positions
DUM = 2048       # dummy table row index
TROWS = 2049     # rows in the 6-graph table
LANES = 512      # table row width (bf16): 6*64 h | 6 es | pad
ESL = 384        # es lane base
HL = 768         # l1out row width (6 x 128-lane chunks, h at chunk base)
NEG = -1e9
LN_EPS = 1e-5
NCORES = 8
SEQ = 500        # LSTM sequences per core


def _ensure_hook():
    """Register the NTFF profile hook if the boot didn't (enables traces)."""
    try:
        try:
            import antenv.axon_hooks  # noqa: F401
        except ImportError:
            import types
            import antenv
            mod = types.ModuleType("antenv.axon_hooks")
            mod._hook = None

            def set_axon_ntff_profile_hook(h, _m=mod):
                _m._hook = h

            def get_axon_ntff_profile_hook(_m=mod):
                return _m._hook

            mod.set_axon_ntff_profile_hook = set_axon_ntff_profile_hook
            mod.get_axon_ntff_profile_hook = get_axon_ntff_profile_hook
            sys.modules["antenv.axon_hooks"] = mod
            antenv.axon_hooks = mod
        from antenv.axon_hooks import (get_axon_ntff_profile_hook,
                                       set_axon_ntff_profile_hook)
        if get_axon_ntff_profile_hook() is None:
            from trn_agent_boot.trn_boot import _ntff_profile_via_ctypes
            so = "/opt/axon/libaxon_pjrt.so"
            if os.path.exists(so):
                set_axon_ntff_profile_hook(_ntff_profile_via_ctypes(so))
    except Exception:
        pass


# ---------------------------------------------------------------------------
# Host-side prep
# ---------------------------------------------------------------------------

def fmt_idx(flat):
    flat = np.asarray(flat)
    assert len(flat) % 128 == 0
    fmt = flat.reshape(-1, 16).T.astype(np.int16)
    return np.tile(fmt, (8, 1))


def prep_structure(edge_src, edge_dst):
    src = np.asarray(edge_src, np.int64)
    dst = np.asarray(edge_dst, np.int64)
    deg = np.bincount(dst, minlength=N)
    pi = np.argsort(deg, kind="stable")
    pi_inv = np.empty(N, np.int64)
    pi_inv[pi] = np.arange(N)

    dpos = pi_inv[dst]
    order = np.argsort(dpos, kind="stable")
    src_s = src[order]
    dpos_s = dpos[order]
    cnt = np.bincount(dpos_s, minlength=NPAD)
    dmax = [int(max(1, cnt[b * 128:(b + 1) * 128].max())) for b in range(NB)]
    coloff = np.concatenate([[0], np.cumsum(dmax)]).astype(np.int64)
    ncol = int(coloff[-1])

    starts = np.concatenate([[0], np.cumsum(cnt)]).astype(np.int64)
    slot = np.full((ncol, 128), DUM, np.int64)
    b_of = dpos_s // 128
    p_of = dpos_s % 128
    jw = np.arange(E) - starts[dpos_s]
    slot[coloff[b_of] + jw, p_of] = pi_inv[src_s]
    eidx = fmt_idx(slot.reshape(-1))

    # LSTM gather rows: (t, local-seq nl) -> lstmtab row node_pi*6 + lg
    per_t = []
    for t in range(T):
        nl = np.arange(SEQ)
        flat_i = nl * T + t
        lg = flat_i // N
        lrow = pi_inv[flat_i % N] * 6 + lg
        lrow = np.concatenate([lrow, np.zeros(12, np.int64)])
        per_t.append(fmt_idx(lrow))  # [128, 32]
    lstmidx = np.ascontiguousarray(np.stack(per_t, axis=1))  # [128, 24, 32]

    return dict(pi=pi, pi_inv=pi_inv, dmax=dmax, coloff=coloff, ncol=ncol,
                eidx=eidx, lstmidx=lstmidx)


def prep_core(st, core):
    pi, pi_inv = st["pi"], st["pi_inv"]
    c = core % 4
    x2rows = np.full((6, NPAD), DUM, np.int64)
    sel = np.zeros((6, NPAD), np.int64)
    ppos = np.arange(N)
    for lg in range(6):
        t2 = 6 * c + lg
        flat_i = pi[ppos] * T + t2
        tp, npr = flat_i // N, flat_i % N
        x2rows[lg, :N] = (tp // 6) * TROWS + pi_inv[npr]
        sel[lg, :N] = tp % 6
    x2idx = np.stack([fmt_idx(x2rows[lg]) for lg in range(6)], axis=1)
    # 5 predicated-select masks per graph (k=1..5); k=0 is the base copy.
    selmask = np.zeros((6, 5, NPAD), np.uint8)
    for lg in range(6):
        for k in range(1, 6):
            selmask[lg, k - 1] = (sel[lg] == k).astype(np.uint8)
    return dict(x2idx=np.ascontiguousarray(x2idx.astype(np.int16)),
                selmask=selmask)


def prep_weights(p):
    f32 = np.float32
    w = {}
    w["fcW"] = np.asarray(p["fc_W"], f32)
    w["fcb"] = np.asarray(p["fc_b"], f32).reshape(64, 1)

    def wc(tag):
        W = np.asarray(p[f"{tag}_W"], f32)
        a_s = np.asarray(p[f"{tag}_asrc"], f32).reshape(64, 1)
        a_d = np.asarray(p[f"{tag}_adst"], f32).reshape(64, 1)
        return np.hstack([W, W @ a_s, W @ a_d]).astype(BF16)  # [64, 66]

    w["W1c"] = wc("g1")
    w["W2c"] = wc("g2")
    w["b1"] = np.asarray(p["g1_b"], f32).reshape(1, 64)
    w["b2"] = np.asarray(p["g2_b"], f32).reshape(1, 64)
    Wih = np.asarray(p["lstm_Wih"], f32)
    Whh = np.asarray(p["lstm_Whh"], f32)
    w["LL"] = np.vstack([Whh.T, Wih.T]).astype(BF16)  # K rows: 0:64=h, 64:128=x
    bihh = np.asarray(p["lstm_bih"], f32) + np.asarray(p["lstm_bhh"], f32)
    w["lstmb"] = np.ascontiguousarray(bihh.reshape(4, 64).T)  # [64,4] i,f,g,o
    w["lng"] = np.asarray(p["ln_g"], f32).reshape(64, 1)
    w["lnb"] = np.asarray(p["ln_b"], f32).reshape(64, 1)
    w["dW"] = np.asarray(p["dense_W"], f32)
    w["db"] = np.asarray(p["dense_b"], f32).reshape(1, 12)
    dum = np.zeros((1, LANES), f32)
    dum[0, ESL:ESL + 6] = NEG
    w["dumrow"] = dum.astype(BF16)
    w["z384"] = np.zeros((1, HL), BF16)
    w["ident"] = np.eye(128, dtype=BF16)
    return w


def prep_xT(p, st, core):
    x = np.asarray(p["x"], np.float32)
    b, c = core // 4, core % 4
    pi = st["pi"]
    xT = np.zeros((NX, 6 * NPAD), np.float32)
    for lg in range(6):
        xT[:, lg * NPAD: lg * NPAD + N] = x[b, pi, 6 * c + lg, :].T
    return np.ascontiguousarray(xT)


# ---------------------------------------------------------------------------
# Bass program
# ---------------------------------------------------------------------------

def build_program(meta):
    import concourse.bass as bass
    import concourse.mybir as mybir
    import concourse.tile as tile
    from concourse import bacc

    dt = mybir.dt
    Alu = mybir.AluOpType
    Act = mybir.ActivationFunctionType
    AX = mybir.AxisListType
    dmax, coloff, ncol = meta["dmax"], meta["coloff"], meta["ncol"]

    nc = bacc.Bacc("TRN2", target_bir_lowering=False, debug=False,
                   num_devices=NCORES)

    def din(name, shape, dty):
        return nc.dram_tensor(name, list(shape), dty, kind="ExternalInput")

    xT_d = din("xT", (NX, 6 * NPAD), dt.float32)
    eidx_d = din("eidx", (128, ncol * 8), dt.int16)
    x2idx_d = din("x2idx", (128, 6, 128), dt.int16)
    lstmidx_d = din("lstmidx", (128, T, 32), dt.int16)
    selmask_d = din("selmask", (6, 5, NPAD), dt.uint8)
    fcW_d = din("fcW", (NX, 64), dt.float32)
    fcb_d = din("fcb", (64, 1), dt.float32)
    W1c_d = din("W1c", (64, 66), dt.bfloat16)
    W2c_d = din("W2c", (64, 66), dt.bfloat16)
    b1_d = din("b1", (1, 64), dt.float32)
    b2_d = din("b2", (1, 64), dt.float32)
    LL_d = din("LL", (128, 256), dt.bfloat16)
    lstmb_d = din("lstmb", (64, 4), dt.float32)
    lng_d = din("lng", (64, 1), dt.float32)
    lnb_d = din("lnb", (64, 1), dt.float32)
    dW_d = din("dW", (64, 12), dt.float32)
    db_d = din("db", (1, 12), dt.float32)
    dum_d = din("dumrow", (1, LANES), dt.bfloat16)
    z384_d = din("z384", (1, HL), dt.bfloat16)
    ident_d = din("ident", (128, 128), dt.bfloat16)
    out_d = nc.dram_tensor("out", [SEQ, 12], dt.float32, kind="ExternalOutput")

    def bcast_ap(dram_t, parts, inner):
        """Partition-broadcast read AP of a DRAM region."""
        ap = dram_t if isinstance(dram_t, bass.AP) else dram_t.ap()
        return bass.AP(tensor=ap.tensor, offset=ap.offset,
                       ap=[[0, parts]] + [list(x) for x in ap.ap[1:]])

    def blast(ap_, n):
        """Append a step-0 dim of size n to an AP (free-dim broadcast)."""
        return bass.AP(tensor=ap_.tensor, offset=ap_.offset,
                       ap=list(ap_.ap) + [[0, n]])

    def midblast(ap_, n):
        """Insert a step-0 dim of size n before the last dim of an AP."""
        return bass.AP(tensor=ap_.tensor, offset=ap_.offset,
                       ap=list(ap_.ap[:-1]) + [[0, n], list(ap_.ap[-1])])

    with tile.TileContext(nc) as tc:
        import contextlib
        ctx = contextlib.ExitStack()
        consts = ctx.enter_context(tc.tile_pool(name="consts", bufs=1))
        dramp = ctx.enter_context(tc.tile_pool(name="dramp", bufs=1,
                                               space="DRAM"))
        rows_p = ctx.enter_context(tc.tile_pool(name="rows", bufs=3))
        msg_p = ctx.enter_context(tc.tile_pool(name="msg", bufs=2))
        wk_p = ctx.enter_context(tc.tile_pool(name="wk", bufs=4))
        wm_p = ctx.enter_context(tc.tile_pool(name="wm", bufs=3))
        edt_p = ctx.enter_context(tc.tile_pool(name="edt", bufs=1))

        # ---- constants into SBUF ----
        def load(name, shape, dty, src_ap):
            t = consts.tile(shape, dty, name=name)
            nc.sync.dma_start(out=t[:], in_=src_ap)
            return t

        eidx_s = load("eidx_s", [128, ncol * 8], dt.int16, eidx_d[:])
        x2idx_s = load("x2idx_s", [128, 6, 128], dt.int16, x2idx_d[:])
        lidx_s = load("lidx_s", [128, T, 32], dt.int16, lstmidx_d[:])
        fcW_s = load("fcW_s", [NX, 64], dt.float32, fcW_d[:])
        fcb_s = load("fcb_s", [64, 1], dt.float32, fcb_d[:])
        W1c_s = load("W1c_s", [64, 66], dt.bfloat16, W1c_d[:])
        W2c_s = load("W2c_s", [64, 66], dt.bfloat16, W2c_d[:])
        LL_s = load("LL_s", [128, 256], dt.bfloat16, LL_d[:])
        dW_s = load("dW_s", [64, 12], dt.float32, dW_d[:])
        lstmb_s = load("lstmb_s", [64, 4], dt.float32, lstmb_d[:])
        lng_s = load("lng_s", [64, 1], dt.float32, lng_d[:])
        lnb_s = load("lnb_s", [64, 1], dt.float32, lnb_d[:])
        ident_s = load("ident_s", [128, 128], dt.bfloat16, ident_d[:])
        bR = {}
        for nm, d_ in (("b1", b1_d), ("b2", b2_d)):
            t = consts.tile([128, 64], dt.float32, name=nm + "R")
            nc.gpsimd.dma_start(out=t[:], in_=bcast_ap(d_, 128, 64))
            bR[nm] = t
        dbR = consts.tile([128, 12], dt.float32, name="dbR")
        nc.gpsimd.dma_start(out=dbR[:], in_=bcast_ap(db_d, 128, 12))
        onesrow = consts.tile([1, 64], dt.float32, name="onesrow")
        nc.vector.memset(onesrow[:], 1.0)
        onescol = consts.tile([128, 1], dt.float32, name="onescol")
        nc.vector.memset(onescol[:], 1.0)
        epsT = consts.tile([1, 1], dt.float32, name="epsT")
        nc.vector.memset(epsT[:], LN_EPS)

        # ---- DRAM tables ----
        htab = {1: dramp.tile([TROWS, LANES], dt.bfloat16, name="htab1"),
                2: dramp.tile([TROWS, LANES], dt.bfloat16, name="htab2")}
        l1out = dramp.tile([TROWS, HL], dt.bfloat16, name="l1out")
        agtab = dramp.tile([4 * TROWS, HL], dt.bfloat16, name="agtab")
        lstmtab = dramp.tile([TROWS, 6, 128], dt.bfloat16, name="lstmtab")

        edt_all = {1: edt_p.tile([128, NB, 6], dt.float32, name="edt1"),
                   2: edt_p.tile([128, NB, 6], dt.float32, name="edt2")}

        # ---- FC: hfcT [64, 12288] bf16 = fcW.T @ xT + fcb ----
        fc_ctx = contextlib.ExitStack()
        hfc_p = fc_ctx.enter_context(tc.tile_pool(name="hfc", bufs=1))
        hfcT = hfc_p.tile([64, 6 * NPAD], dt.bfloat16, name="hfcT")
        with tc.tile_pool(name="xtp", bufs=1) as xt_p, \
             tc.tile_pool(name="ps_fc", bufs=2, space="PSUM") as ps_fc:
            xT_s = xt_p.tile([NX, 6 * NPAD], dt.float32, name="xT_s")
            nc.sync.dma_start(out=xT_s[:], in_=xT_d[:])
            for chix in range(6 * NPAD // 512):
                sl = slice(chix * 512, (chix + 1) * 512)
                ps = ps_fc.tile([64, 512], dt.float32, space="PSUM",
                                tag="psfc")
                nc.tensor.matmul(ps[:], lhsT=fcW_s[:], rhs=xT_s[:, sl],
                                 start=True, stop=True)
                nc.vector.tensor_scalar_add(hfcT[:, sl], ps[:], fcb_s[:])

        def table_build(layer, lhsT_of):
            """Build the 6-graph h-table + edt for one layer."""
            Wc = W1c_s if layer == 1 else W2c_s
            edt = edt_all[layer]
            for chn in range(NB):
                rowt = rows_p.tile([128, LANES], dt.bfloat16, tag="rowt")
                for g in range(6):
                    ps = ps_h.tile([128, 66], dt.float32, space="PSUM",
                                   tag="psh")
                    nc.tensor.matmul(ps[:], lhsT=lhsT_of(g, chn), rhs=Wc[:],
                                     start=True, stop=True)
                    nc.scalar.activation(rowt[:, g * 64:(g + 1) * 64],
                                         ps[:, 0:64], Act.Copy)
                    nc.vector.tensor_copy(rowt[:, ESL + g:ESL + g + 1],
                                          ps[:, 64:65])
                    nc.vector.tensor_copy(edt[:, chn, g:g + 1],
                                          ps[:, 65:66])
                nc.sync.dma_start(
                    out=htab[layer][chn * 128:(chn + 1) * 128, 0:ESL + 6],
                    in_=rowt[:, 0:ESL + 6])
            nc.sync.dma_start(out=htab[layer][DUM:DUM + 1, :], in_=dum_d[:])

        def sparse_phase(layer):
            """Per-block gather + GAT softmax/aggregate, all 6 graphs."""
            bias = bR["b1"] if layer == 1 else bR["b2"]
            edt = edt_all[layer]
            for b in range(NB):
                D = dmax[b]
                msg = msg_p.tile([128, D, LANES], dt.bfloat16, tag="msg")
                nc.gpsimd.dma_gather(
                    msg[:], htab[layer],
                    eidx_s[:, 8 * int(coloff[b]): 8 * int(coloff[b] + D)],
                    128 * D, 128 * D, LANES, single_packet=False)
                # est [128, D, 6] = es(src) + ed(dst)
                est = wk_p.tile([128, D, 6], dt.float32, tag="est")
                edt_b = bass.AP(tensor=edt[:].tensor,
                                offset=edt[:].offset + b * 6,
                                ap=[list(edt[:].ap[0]), [0, D], [1, 6]])
                nc.vector.tensor_tensor(out=est[:],
                                        in0=msg[:, :, ESL:ESL + 6],
                                        in1=edt_b, op=Alu.add)
                estL = wk_p.tile([128, D * 6], dt.float32, tag="estL")
                ef = est[:].rearrange("p d g -> p (d g)")
                nc.vector.scalar_tensor_tensor(out=estL[:], in0=ef,
                                               scalar=0.2, in1=ef,
                                               op0=Alu.mult, op1=Alu.max)
                wbf = wk_p.tile([128, D, 6], dt.bfloat16, tag="wbf")
                nc.scalar.activation(
                    wbf[:].rearrange("p d g -> p (d g)"), estL[:], Act.Exp)
                den = wk_p.tile([128, 6], dt.float32, tag="den")
                nc.vector.tensor_reduce(den[:],
                                        wbf[:].rearrange("p d g -> p g d"),
                                        axis=AX.X, op=Alu.add)
                rcol = wk_p.tile([128, 6], dt.float32, tag="rcol")
                nc.vector.reciprocal(rcol[:], den[:])
                psum6 = ps_agg.tile([128, 6, 64], dt.float32, space="PSUM",
                                    tag="psagg")
                for g in range(6):
                    wm = wm_p.tile([128, D, 64], dt.bfloat16, tag="wm")
                    wbf_g = bass.AP(tensor=wbf[:].tensor,
                                    offset=wbf[:].offset + g,
                                    ap=[list(wbf[:].ap[0]), [6, D], [0, 64]])
                    nc.vector.tensor_tensor(out=wm[:],
                                            in0=msg[:, :, g * 64:(g + 1) * 64],
                                            in1=wbf_g, op=Alu.mult)
                    for c in range(D):
                        nc.tensor.matmul(psum6[:, g, :], lhsT=ident_s[:],
                                         rhs=wm[:, c, :],
                                         start=(c == 0), stop=(c == D - 1))
                if layer == 1:
                    outrow = rows_p.tile([128, HL], dt.bfloat16, tag="orow")
                else:
                    lrow6 = rows_p.tile([128, 6, 128], dt.bfloat16,
                                        tag="lrow6")
                    nc.vector.memset(lrow6[:, :, 0:64], 0.0)
                for g in range(6):
                    gout = wk_p.tile([128, 64], dt.float32, tag="gout")
                    nc.vector.scalar_tensor_tensor(
                        out=gout[:], in0=psum6[:, g, :],
                        scalar=rcol[:, g:g + 1], in1=bias[:],
                        op0=Alu.mult, op1=Alu.add)
                    if layer == 1:
                        nc.scalar.activation(outrow[:, g * 128:g * 128 + 64],
                                             gout[:], Act.Gelu)
                    else:
                        nc.scalar.activation(lrow6[:, g, 64:128],
                                             gout[:], Act.Gelu)
                if layer == 1:
                    nc.sync.dma_start(
                        out=l1out[b * 128:(b + 1) * 128, :], in_=outrow[:])
                else:
                    nc.sync.dma_start(
                        out=lstmtab[b * 128:(b + 1) * 128, :, :],
                        in_=lrow6[:])
            if layer == 1:
                nc.sync.dma_start(out=l1out[DUM:DUM + 1, :], in_=z384_d[:])

        gcn_ctx = contextlib.ExitStack()
        ps_h = gcn_ctx.enter_context(tc.tile_pool(name="ps_h", bufs=2,
                                                  space="PSUM"))
        ps_agg = gcn_ctx.enter_context(tc.tile_pool(name="ps_agg", bufs=2,
                                                    space="PSUM"))

        # ---- layer 1 ----
        def l1_lhsT(g, chn):
            return hfcT[:, g * NPAD + chn * 128: g * NPAD + (chn + 1) * 128]
        table_build(1, l1_lhsT)
        sparse_phase(1)
        fc_ctx.close()

        # ---- allgather ----
        nc.gpsimd.collective_compute(
            "AllGather", Alu.bypass,
            replica_groups=[[0, 1, 2, 3], [4, 5, 6, 7]],
            ins=[l1out[:].opt()], outs=[agtab[:].opt()])

        # ---- layer 2 input: gather + 6-way select ----
        l2_ctx = contextlib.ExitStack()
        x2_p = l2_ctx.enter_context(tc.tile_pool(name="x2", bufs=2))
        x2s_p = l2_ctx.enter_context(tc.tile_pool(name="x2s", bufs=1))
        mask_p = l2_ctx.enter_context(tc.tile_pool(name="mask", bufs=2))
        x2sel = []
        for g in range(6):
            xg = x2_p.tile([128, 6, NPAD], dt.bfloat16, tag="xg")
            nc.gpsimd.dma_gather(xg[:], agtab[:], x2idx_s[:, g, :],
                                 NPAD, NPAD, HL, transpose=True,
                                 single_packet=False)
            msk = mask_p.tile([64, 5, NPAD], dt.uint8, tag="msk")
            nc.gpsimd.dma_start(
                out=msk[:],
                in_=bass.AP(tensor=selmask_d.ap().tensor,
                            offset=g * 5 * NPAD,
                            ap=[[0, 64], [NPAD, 5], [1, NPAD]]))
            xs = x2s_p.tile([64, NPAD], dt.bfloat16, name=f"xs{g}")
            nc.vector.tensor_copy(xs[:], xg[0:64, 0, :])
            for k in range(1, 6):
                nc.vector.copy_predicated(
                    xs[:], msk[:, k - 1, :], xg[0:64, k, :])
            x2sel.append(xs)

        # ---- layer 2 ----
        def l2_lhsT(g, chn):
            return x2sel[g][:, chn * 128:(chn + 1) * 128]
        table_build(2, l2_lhsT)
        sparse_phase(2)
        l2_ctx.close()
        gcn_ctx.close()

        # ---- LSTM ----
        lst_ctx = contextlib.ExitStack()
        big = lst_ctx.enter_context(tc.tile_pool(name="big", bufs=1))
        lst_p = lst_ctx.enter_context(tc.tile_pool(name="lst", bufs=2))
        ps_misc = lst_ctx.enter_context(tc.tile_pool(name="ps_misc", bufs=1,
                                                     space="PSUM"))
        ps_z = lst_ctx.enter_context(tc.tile_pool(name="ps_z", bufs=4,
                                                  space="PSUM"))
        lt = lstmtab[:]
        lt_flat = bass.AP(tensor=lt.tensor, offset=lt.offset,
                          ap=[[128, TROWS * 6], [1, 128]])
        XTs = []
        for tg in range(T):
            XT = big.tile([128, 1, 512], dt.bfloat16, name=f"XT{tg}")
            nc.gpsimd.dma_gather(XT[:], lt_flat,
                                 lidx_s[:, tg, :], 512, 500, 128,
                                 transpose=True, single_packet=False)
            XTs.append(XT)
        stacked = big.tile([128, SEQ], dt.bfloat16, name="stacked")
        cT = big.tile([64, SEQ], dt.float32, name="cT")
        h23 = big.tile([64, SEQ], dt.float32, name="h23")
        nc.vector.memset(stacked[0:64, :], 0.0)
        nc.vector.memset(cT[:], 0.0)
        for t in range(T):
            nc.vector.tensor_copy(stacked[64:128, :], XTs[t][64:128, 0, 0:SEQ])
            zs = []
            for g in range(4):
                ps = ps_z.tile([64, SEQ], dt.float32, space="PSUM",
                               tag="pslstm", name=f"z{g}")
                nc.tensor.matmul(ps[:], lhsT=LL_s[:, g * 64:(g + 1) * 64],
                                 rhs=stacked[:], start=True, stop=True)
                zs.append(ps)
            gates = [None] * 4
            for g, fn in ((0, Act.Sigmoid), (1, Act.Sigmoid),
                          (3, Act.Sigmoid), (2, Act.Tanh)):
                gt = lst_p.tile([64, SEQ], dt.float32, tag=f"gate{g}",
                                name=f"gate{g}")
                nc.scalar.activation(gt[:], zs[g][:], fn,
                                     bias=lstmb_s[:, g:g + 1], scale=1.0)
                gates[g] = gt
            ig = lst_p.tile([64, SEQ], dt.float32, tag="ig")
            nc.vector.tensor_mul(ig[:], gates[0][:], gates[2][:])
            nc.vector.tensor_mul(cT[:], gates[1][:], cT[:])
            nc.vector.tensor_add(cT[:], cT[:], ig[:])
            th = lst_p.tile([64, SEQ], dt.float32, tag="th")
            nc.scalar.activation(th[:], cT[:], Act.Tanh)
            if t < T - 1:
                nc.vector.tensor_mul(stacked[0:64, :], gates[3][:], th[:])
            else:
                nc.vector.tensor_mul(h23[:], gates[3][:], th[:])

        # ---- LayerNorm (transposed; stats via ones-matmuls) ----
        ps_mu = ps_misc.tile([1, SEQ], dt.float32, space="PSUM", tag="psmisc",
                             name="ps_mu")
        nc.tensor.matmul(ps_mu[:], lhsT=onescol[0:64, :], rhs=h23[:],
                         start=True, stop=True)
        mu_sb = lst_p.tile([1, SEQ], dt.float32, tag="mu")
        nc.scalar.activation(mu_sb[:], ps_mu[:], Act.Copy, scale=1.0 / 64)
        ps_mub = ps_misc.tile([64, SEQ], dt.float32, space="PSUM",
                              tag="psb500", name="ps_mub")
        nc.tensor.matmul(ps_mub[:], lhsT=onesrow[:], rhs=mu_sb[:],
                         start=True, stop=True)
        dtl = lst_p.tile([64, SEQ], dt.float32, tag="dtl")
        nc.vector.tensor_sub(dtl[:], h23[:], ps_mub[:])
        sq = lst_p.tile([64, SEQ], dt.float32, tag="sq")
        nc.vector.tensor_mul(sq[:], dtl[:], dtl[:])
        ps_var = ps_misc.tile([1, SEQ], dt.float32, space="PSUM", tag="psmisc",
                              name="ps_var")
        nc.tensor.matmul(ps_var[:], lhsT=onescol[0:64, :], rhs=sq[:],
                         start=True, stop=True)
        sd_sb = lst_p.tile([1, SEQ], dt.float32, tag="sd")
        nc.scalar.activation(sd_sb[:], ps_var[:], Act.Sqrt, bias=epsT[:],
                             scale=1.0 / 64)
        rstd = lst_p.tile([1, SEQ], dt.float32, tag="rstd")
        nc.vector.reciprocal(rstd[:], sd_sb[:])
        ps_rb = ps_misc.tile([64, SEQ], dt.float32, space="PSUM",
                             tag="psb500", name="ps_rb")
        nc.tensor.matmul(ps_rb[:], lhsT=onesrow[:], rhs=rstd[:],
                         start=True, stop=True)
        hn = lst_p.tile([64, SEQ], dt.float32, tag="hn")
        nc.vector.tensor_mul(hn[:], dtl[:], ps_rb[:])
        nc.vector.tensor_scalar(out=hn[:], in0=hn[:],
                                scalar1=lng_s[:], scalar2=lnb_s[:],
                                op0=Alu.mult, op1=Alu.add)
        # ---- dense ----
        for q in range(4):
            cs = min(128, SEQ - q * 128)
            psd = ps_misc.tile([128, 12], dt.float32, space="PSUM",
                               tag="psmisc", name="psd")
            nc.tensor.matmul(psd[0:cs, :],
                             lhsT=hn[:, q * 128:q * 128 + cs],
                             rhs=dW_s[:], start=True, stop=True)
            ot = lst_p.tile([128, 12], dt.float32, tag="ot")
            nc.vector.tensor_add(ot[0:cs, :], psd[0:cs, :], dbR[0:cs, :])
            nc.sync.dma_start(out=out_d[q * 128:q * 128 + cs, :],
                              in_=ot[0:cs, :])
        lst_ctx.close()
        ctx.close()

    nc.compile()
    return nc


# ---------------------------------------------------------------------------
# Runner
# ---------------------------------------------------------------------------

_CACHE = {}


def _get_program_and_maps(inputs):
    st = prep_structure(inputs["edge_src"], inputs["edge_dst"])
    w = prep_weights(inputs)
    meta = dict(dmax=st["dmax"], coloff=st["coloff"], ncol=st["ncol"])
    key = ("prog", tuple(st["dmax"]))
    if key not in _CACHE:
        _CACHE[key] = build_program(meta)
    nc = _CACHE[key]

    shared = dict(eidx=st["eidx"], lstmidx=st["lstmidx"], fcW=w["fcW"],
                  fcb=w["fcb"], W1c=w["W1c"], W2c=w["W2c"],
                  b1=w["b1"], b2=w["b2"], LL=w["LL"], lstmb=w["lstmb"],
                  lng=w["lng"], lnb=w["lnb"], dW=w["dW"], db=w["db"],
                  dumrow=w["dumrow"], z384=w["z384"], ident=w["ident"])
    in_maps = []
    for core in range(NCORES):
        pc = prep_core(st, core)
        m = dict(shared)
        m["xT"] = prep_xT(inputs, st, core)
        m["x2idx"] = pc["x2idx"]
        m["selmask"] = pc["selmask"]
        in_maps.append(m)
    return nc, in_maps


def run_on_hw(inputs, trace=False):
    _ensure_hook()
    from concourse.bass_utils import run_bass_kernel_spmd
    nc, in_maps = _get_program_and_maps(inputs)
    res = run_bass_kernel_spmd(nc, in_maps, list(range(NCORES)), trace=trace)
    out_full = np.zeros((B, N, PL, 1), np.float32)
    for core in range(NCORES):
        bb, c = core // 4, core % 4
        out_full[bb, 500 * c:500 * (c + 1), :, 0] = res.results[core]["out"]
    return out_full, res


def kernel(x, edge_src, edge_dst, fc_W, fc_b,
           g1_W, g1_b, g1_asrc, g1_adst,
           g2_W, g2_b, g2_asrc, g2_adst,
           lstm_Wih, lstm_Whh, lstm_bih, lstm_bhh,
           ln_g, ln_b, dense_W, dense_b):
    inputs = dict(x=x, edge_src=edge_src, edge_dst=edge_dst, fc_W=fc_W,
                  fc_b=fc_b, g1_W=g1_W, g1_b=g1_b, g1_asrc=g1_asrc,
                  g1_adst=g1_adst, g2_W=g2_W, g2_b=g2_b, g2_asrc=g2_asrc,
                  g2_adst=g2_adst, lstm_Wih=lstm_Wih, lstm_Whh=lstm_Whh,
                  lstm_bih=lstm_bih, lstm_bhh=lstm_bhh, ln_g=ln_g, ln_b=ln_b,
                  dense_W=dense_W, dense_b=dense_b)
    out, _ = run_on_hw(inputs, trace=False)
    return out


# revision 25
# speedup vs baseline: 799.9740x; 1.0728x over previous
"""GCN(GAT)x2 + LSTM + LN + dense on 8 Trainium2 NeuronCores (Bass/Tile).

V2: data-parallel over B*T=48 graphs (6 per core). All 6 graphs of a core
pack into ONE h-table (1KB rows: 6x64 h lanes + 6 es lanes), so the
per-edge slot gather costs one Q7 descriptor stream per layer instead of
three. es/ed come from the same matmul as h (rhs = [W | W@a_s | W@a_d]).
Softmax drops the max-shift (est bounded ~|6.5|, exp safe in f32). The
per-edge weighted slot-sum runs on the Tensor engine as identity-matmul
PSUM accumulation; DVE only does the est chain + one blast-multiply per
graph. LSTM stage unchanged (core-local via the scramble block structure).
"""

import os
import sys
import numpy as np
import ml_dtypes

BF16 = ml_dtypes.bfloat16

N = 2000
NPAD = 2048
T = 24
B = 2
NX = 8
H = 64
E = 32000
PL = 12
NB = 16          # dst blocks of 128 pi-positions
DUM = 2048       # dummy table row index
TROWS = 2049     # rows in the 6-graph table
LANES = 512      # table row width (bf16): 6*64 h | 6 es | pad
ESL = 384        # es lane base
HL = 768         # l1out row width (6 x 128-lane chunks, h at chunk base)
NEG = -1e9
LN_EPS = 1e-5
NCORES = 8
SEQ = 500        # LSTM sequences per core


def _ensure_hook():
    """Register the NTFF profile hook if the boot didn't (enables traces)."""
    try:
        try:
            import antenv.axon_hooks  # noqa: F401
        except ImportError:
            import types
            import antenv
            mod = types.ModuleType("antenv.axon_hooks")
            mod._hook = None

            def set_axon_ntff_profile_hook(h, _m=mod):
                _m._hook = h

            def get_axon_ntff_profile_hook(_m=mod):
                return _m._hook

            mod.set_axon_ntff_profile_hook = set_axon_ntff_profile_hook
            mod.get_axon_ntff_profile_hook = get_axon_ntff_profile_hook
            sys.modules["antenv.axon_hooks"] = mod
            antenv.axon_hooks = mod
        from antenv.axon_hooks import (get_axon_ntff_profile_hook,
                                       set_axon_ntff_profile_hook)
        if get_axon_ntff_profile_hook() is None:
            from trn_agent_boot.trn_boot import _ntff_profile_via_ctypes
            so = "/opt/axon/libaxon_pjrt.so"
            if os.path.exists(so):
                set_axon_ntff_profile_hook(_ntff_profile_via_ctypes(so))
    except Exception:
        pass


# ---------------------------------------------------------------------------
# Host-side prep
# ---------------------------------------------------------------------------

def fmt_idx(flat):
    flat = np.asarray(flat)
    assert len(flat) % 128 == 0
    fmt = flat.reshape(-1, 16).T.astype(np.int16)
    return np.tile(fmt, (8, 1))


def prep_structure(edge_src, edge_dst):
    src = np.asarray(edge_src, np.int64)
    dst = np.asarray(edge_dst, np.int64)
    deg = np.bincount(dst, minlength=N)
    pi = np.argsort(deg, kind="stable")
    pi_inv = np.empty(N, np.int64)
    pi_inv[pi] = np.arange(N)

    dpos = pi_inv[dst]
    order = np.argsort(dpos, kind="stable")
    src_s = src[order]
    dpos_s = dpos[order]
    cnt = np.bincount(dpos_s, minlength=NPAD)
    dmax = [int(max(1, cnt[b * 128:(b + 1) * 128].max())) for b in range(NB)]
    coloff = np.concatenate([[0], np.cumsum(dmax)]).astype(np.int64)
    ncol = int(coloff[-1])

    starts = np.concatenate([[0], np.cumsum(cnt)]).astype(np.int64)
    slot = np.full((ncol, 128), DUM, np.int64)
    b_of = dpos_s // 128
    p_of = dpos_s % 128
    jw = np.arange(E) - starts[dpos_s]
    slot[coloff[b_of] + jw, p_of] = pi_inv[src_s]
    eidx = fmt_idx(slot.reshape(-1))

    # LSTM gather rows: (t, local-seq nl) -> lstmtab row node_pi*6 + lg
    per_t = []
    for t in range(T):
        nl = np.arange(SEQ)
        flat_i = nl * T + t
        lg = flat_i // N
        lrow = pi_inv[flat_i % N] * 6 + lg
        lrow = np.concatenate([lrow, np.zeros(12, np.int64)])
        per_t.append(fmt_idx(lrow))  # [128, 32]
    lstmidx = np.ascontiguousarray(np.stack(per_t, axis=1))  # [128, 24, 32]

    return dict(pi=pi, pi_inv=pi_inv, dmax=dmax, coloff=coloff, ncol=ncol,
                eidx=eidx, lstmidx=lstmidx)


def prep_core(st, core):
    pi, pi_inv = st["pi"], st["pi_inv"]
    c = core % 4
    x2rows = np.full((6, NPAD), DUM, np.int64)
    sel = np.zeros((6, NPAD), np.int64)
    ppos = np.arange(N)
    for lg in range(6):
        t2 = 6 * c + lg
        flat_i = pi[ppos] * T + t2
        tp, npr = flat_i // N, flat_i % N
        # agtab row: [chunk(4 blocks), src-core, row-in-chunk]
        q = pi_inv[npr] // 512
        x2rows[lg, :N] = q * 2048 + (tp // 6) * 512 + pi_inv[npr] % 512
        sel[lg, :N] = tp % 6
    x2rows[:, N:] = 0
    x2idx = np.stack([fmt_idx(x2rows[lg]) for lg in range(6)], axis=1)
    # 5 predicated-select masks per graph (k=1..5); k=0 is the base copy.
    selmask = np.zeros((6, 5, NPAD), np.uint8)
    for lg in range(6):
        for k in range(1, 6):
            selmask[lg, k - 1] = (sel[lg] == k).astype(np.uint8)
    return dict(x2idx=np.ascontiguousarray(x2idx.astype(np.int16)),
                selmask=selmask)


def prep_weights(p):
    f32 = np.float32
    w = {}
    w["fcW"] = np.asarray(p["fc_W"], f32).astype(BF16)
    w["fcb"] = np.asarray(p["fc_b"], f32).reshape(64, 1)

    def wc(tag):
        W = np.asarray(p[f"{tag}_W"], f32)
        a_s = np.asarray(p[f"{tag}_asrc"], f32).reshape(64, 1)
        a_d = np.asarray(p[f"{tag}_adst"], f32).reshape(64, 1)
        return np.hstack([W, W @ a_s, W @ a_d]).astype(BF16)  # [64, 66]

    w["W1c"] = wc("g1")
    w["W2c"] = wc("g2")
    w["b1"] = np.asarray(p["g1_b"], f32).reshape(1, 64)
    w["b2"] = np.asarray(p["g2_b"], f32).reshape(1, 64)
    Wih = np.asarray(p["lstm_Wih"], f32)
    Whh = np.asarray(p["lstm_Whh"], f32)
    w["LL"] = np.vstack([Whh.T, Wih.T]).astype(BF16)  # K rows: 0:64=h, 64:128=x
    bihh = np.asarray(p["lstm_bih"], f32) + np.asarray(p["lstm_bhh"], f32)
    w["lstmb"] = np.ascontiguousarray(bihh.reshape(4, 64).T)  # [64,4] i,f,g,o
    w["lng"] = np.asarray(p["ln_g"], f32).reshape(64, 1)
    w["lnb"] = np.asarray(p["ln_b"], f32).reshape(64, 1)
    w["dW"] = np.asarray(p["dense_W"], f32)
    w["db"] = np.asarray(p["dense_b"], f32).reshape(1, 12)
    dum = np.zeros((1, LANES), f32)
    dum[0, ESL:ESL + 6] = NEG
    w["dumrow"] = dum.astype(BF16)
    w["z384"] = np.zeros((1, HL), BF16)
    w["ident"] = np.eye(128, dtype=BF16)
    return w


def prep_xT(p, st, core):
    x = np.asarray(p["x"], np.float32)
    b, c = core // 4, core % 4
    pi = st["pi"]
    xT = np.zeros((NX, 6 * NPAD), np.float32)
    for lg in range(6):
        xT[:, lg * NPAD: lg * NPAD + N] = x[b, pi, 6 * c + lg, :].T
    return np.ascontiguousarray(xT.astype(BF16))


# ---------------------------------------------------------------------------
# Bass program
# ---------------------------------------------------------------------------

def build_program(meta):
    import concourse.bass as bass
    import concourse.mybir as mybir
    import concourse.tile as tile
    from concourse import bacc

    dt = mybir.dt
    Alu = mybir.AluOpType
    Act = mybir.ActivationFunctionType
    AX = mybir.AxisListType
    dmax, coloff, ncol = meta["dmax"], meta["coloff"], meta["ncol"]

    nc = bacc.Bacc("TRN2", target_bir_lowering=False, debug=False,
                   num_devices=NCORES)

    def din(name, shape, dty):
        return nc.dram_tensor(name, list(shape), dty, kind="ExternalInput")

    xT_d = din("xT", (NX, 6 * NPAD), dt.bfloat16)
    eidx_d = din("eidx", (128, ncol * 8), dt.int16)
    x2idx_d = din("x2idx", (128, 6, 128), dt.int16)
    lstmidx_d = din("lstmidx", (128, T, 32), dt.int16)
    selmask_d = din("selmask", (6, 5, NPAD), dt.uint8)
    fcW_d = din("fcW", (NX, 64), dt.bfloat16)
    fcb_d = din("fcb", (64, 1), dt.float32)
    W1c_d = din("W1c", (64, 66), dt.bfloat16)
    W2c_d = din("W2c", (64, 66), dt.bfloat16)
    b1_d = din("b1", (1, 64), dt.float32)
    b2_d = din("b2", (1, 64), dt.float32)
    LL_d = din("LL", (128, 256), dt.bfloat16)
    lstmb_d = din("lstmb", (64, 4), dt.float32)
    lng_d = din("lng", (64, 1), dt.float32)
    lnb_d = din("lnb", (64, 1), dt.float32)
    dW_d = din("dW", (64, 12), dt.float32)
    db_d = din("db", (1, 12), dt.float32)
    dum_d = din("dumrow", (1, LANES), dt.bfloat16)
    z384_d = din("z384", (1, HL), dt.bfloat16)
    ident_d = din("ident", (128, 128), dt.bfloat16)
    out_d = nc.dram_tensor("out", [SEQ, 12], dt.float32, kind="ExternalOutput")

    def bcast_ap(dram_t, parts, inner):
        """Partition-broadcast read AP of a DRAM region."""
        ap = dram_t if isinstance(dram_t, bass.AP) else dram_t.ap()
        return bass.AP(tensor=ap.tensor, offset=ap.offset,
                       ap=[[0, parts]] + [list(x) for x in ap.ap[1:]])

    def blast(ap_, n):
        """Append a step-0 dim of size n to an AP (free-dim broadcast)."""
        return bass.AP(tensor=ap_.tensor, offset=ap_.offset,
                       ap=list(ap_.ap) + [[0, n]])

    def midblast(ap_, n):
        """Insert a step-0 dim of size n before the last dim of an AP."""
        return bass.AP(tensor=ap_.tensor, offset=ap_.offset,
                       ap=list(ap_.ap[:-1]) + [[0, n], list(ap_.ap[-1])])

    with tile.TileContext(nc) as tc:
        import contextlib
        ctx = contextlib.ExitStack()
        consts = ctx.enter_context(tc.tile_pool(name="consts", bufs=1))
        dramp = ctx.enter_context(tc.tile_pool(name="dramp", bufs=1,
                                               space="DRAM"))
        rows_p = ctx.enter_context(tc.tile_pool(name="rows", bufs=3))
        msg_p = ctx.enter_context(tc.tile_pool(name="msg", bufs=2))
        wk_p = ctx.enter_context(tc.tile_pool(name="wk", bufs=4))
        wm_p = ctx.enter_context(tc.tile_pool(name="wm", bufs=3))
        edt_p = ctx.enter_context(tc.tile_pool(name="edt", bufs=1))

        # ---- constants into SBUF ----
        def load(name, shape, dty, src_ap):
            t = consts.tile(shape, dty, name=name)
            nc.sync.dma_start(out=t[:], in_=src_ap)
            return t

        eidx_s = load("eidx_s", [128, ncol * 8], dt.int16, eidx_d[:])
        x2idx_s = load("x2idx_s", [128, 6, 128], dt.int16, x2idx_d[:])
        lidx_s = load("lidx_s", [128, T, 32], dt.int16, lstmidx_d[:])
        fcW_s = load("fcW_s", [NX, 64], dt.bfloat16, fcW_d[:])
        fcb_s = load("fcb_s", [64, 1], dt.float32, fcb_d[:])
        W1c_s = load("W1c_s", [64, 66], dt.bfloat16, W1c_d[:])
        W2c_s = load("W2c_s", [64, 66], dt.bfloat16, W2c_d[:])
        LL_s = load("LL_s", [128, 256], dt.bfloat16, LL_d[:])
        dW_s = load("dW_s", [64, 12], dt.float32, dW_d[:])
        lstmb_s = load("lstmb_s", [64, 4], dt.float32, lstmb_d[:])
        lng_s = load("lng_s", [64, 1], dt.float32, lng_d[:])
        lnb_s = load("lnb_s", [64, 1], dt.float32, lnb_d[:])
        ident_s = load("ident_s", [128, 128], dt.bfloat16, ident_d[:])
        bR = {}
        for nm, d_ in (("b1", b1_d), ("b2", b2_d)):
            t = consts.tile([128, 64], dt.float32, name=nm + "R")
            nc.gpsimd.dma_start(out=t[:], in_=bcast_ap(d_, 128, 64))
            bR[nm] = t
        dbR = consts.tile([128, 12], dt.float32, name="dbR")
        nc.gpsimd.dma_start(out=dbR[:], in_=bcast_ap(db_d, 128, 12))
        onesrow = consts.tile([1, 64], dt.float32, name="onesrow")
        nc.vector.memset(onesrow[:], 1.0)
        onescol = consts.tile([128, 1], dt.float32, name="onescol")
        nc.vector.memset(onescol[:], 1.0)
        epsT = consts.tile([1, 1], dt.float32, name="epsT")
        nc.vector.memset(epsT[:], LN_EPS)

        # ---- DRAM tables ----
        htab = {1: dramp.tile([TROWS, LANES], dt.bfloat16, name="htab1"),
                2: dramp.tile([TROWS, LANES], dt.bfloat16, name="htab2")}
        # l1out in 4 chunks of 4 dst-blocks so each AllGather chunk can
        # launch as soon as its blocks are done (slice-level dep via tiles).
        l1out = [dramp.tile([512, HL], dt.bfloat16, name=f"l1out{q}")
                 for q in range(4)]
        agtab = dramp.tile([4 * 2048, HL], dt.bfloat16, name="agtab")
        lstmtab = dramp.tile([TROWS, 6, 128], dt.bfloat16, name="lstmtab")

        edt_all = {1: edt_p.tile([128, NB, 6], dt.float32, name="edt1"),
                   2: edt_p.tile([128, NB, 6], dt.float32, name="edt2")}

        # ---- FC: hfcT [64, 12288] bf16 = fcW.T @ xT + fcb ----
        fc_ctx = contextlib.ExitStack()
        hfc_p = fc_ctx.enter_context(tc.tile_pool(name="hfc", bufs=1))
        hfcT = hfc_p.tile([64, 6 * NPAD], dt.bfloat16, name="hfcT")
        with tc.tile_pool(name="xtp", bufs=1) as xt_p, \
             tc.tile_pool(name="ps_fc", bufs=2, space="PSUM") as ps_fc:
            xT_s = xt_p.tile([NX, 6 * NPAD], dt.bfloat16, name="xT_s")
            nc.sync.dma_start(out=xT_s[:], in_=xT_d[:])
            for chix in range(6 * NPAD // 512):
                sl = slice(chix * 512, (chix + 1) * 512)
                ps = ps_fc.tile([64, 512], dt.float32, space="PSUM",
                                tag="psfc")
                nc.tensor.matmul(ps[:], lhsT=fcW_s[:], rhs=xT_s[:, sl],
                                 start=True, stop=True)
                nc.vector.tensor_scalar_add(hfcT[:, sl], ps[:], fcb_s[:])

        def table_build(layer, lhsT_of):
            """Build the 6-graph h-table + edt for one layer."""
            Wc = W1c_s if layer == 1 else W2c_s
            edt = edt_all[layer]
            for chn in range(NB):
                rowt = rows_p.tile([128, LANES], dt.bfloat16, tag="rowt")
                for g in range(6):
                    ps = ps_h.tile([128, 66], dt.float32, space="PSUM",
                                   tag="psh")
                    nc.tensor.matmul(ps[:], lhsT=lhsT_of(g, chn), rhs=Wc[:],
                                     start=True, stop=True)
                    nc.scalar.activation(rowt[:, g * 64:(g + 1) * 64],
                                         ps[:, 0:64], Act.Copy)
                    nc.vector.tensor_copy(rowt[:, ESL + g:ESL + g + 1],
                                          ps[:, 64:65])
                    nc.vector.tensor_copy(edt[:, chn, g:g + 1],
                                          ps[:, 65:66])
                nc.sync.dma_start(
                    out=htab[layer][chn * 128:(chn + 1) * 128, 0:ESL + 6],
                    in_=rowt[:, 0:ESL + 6])
            nc.sync.dma_start(out=htab[layer][DUM:DUM + 1, :], in_=dum_d[:])

        def sparse_phase(layer, post_block=None):
            """Per-block gather + GAT softmax/aggregate, all 6 graphs."""
            bias = bR["b1"] if layer == 1 else bR["b2"]
            edt = edt_all[layer]
            for b in range(NB):
                D = dmax[b]
                msg = msg_p.tile([128, D, LANES], dt.bfloat16, tag="msg")
                nc.gpsimd.dma_gather(
                    msg[:], htab[layer],
                    eidx_s[:, 8 * int(coloff[b]): 8 * int(coloff[b] + D)],
                    128 * D, 128 * D, LANES, single_packet=False)
                # est [128, D, 6] = es(src) + ed(dst)
                est = wk_p.tile([128, D, 6], dt.float32, tag="est")
                edt_b = bass.AP(tensor=edt[:].tensor,
                                offset=edt[:].offset + b * 6,
                                ap=[list(edt[:].ap[0]), [0, D], [1, 6]])
                nc.vector.tensor_tensor(out=est[:],
                                        in0=msg[:, :, ESL:ESL + 6],
                                        in1=edt_b, op=Alu.add)
                estL = wk_p.tile([128, D * 6], dt.float32, tag="estL")
                ef = est[:].rearrange("p d g -> p (d g)")
                nc.vector.scalar_tensor_tensor(out=estL[:], in0=ef,
                                               scalar=0.2, in1=ef,
                                               op0=Alu.mult, op1=Alu.max)
                wbf = wk_p.tile([128, D, 6], dt.bfloat16, tag="wbf")
                nc.scalar.activation(
                    wbf[:].rearrange("p d g -> p (d g)"), estL[:], Act.Exp)
                den = wk_p.tile([128, 6], dt.float32, tag="den")
                nc.vector.tensor_reduce(den[:],
                                        wbf[:].rearrange("p d g -> p g d"),
                                        axis=AX.X, op=Alu.add)
                rcol = wk_p.tile([128, 6], dt.float32, tag="rcol")
                nc.vector.reciprocal(rcol[:], den[:])
                psum6 = ps_agg.tile([128, 6, 64], dt.float32, space="PSUM",
                                    tag="psagg")
                for g in range(6):
                    wm = wm_p.tile([128, D, 64], dt.bfloat16, tag="wm")
                    wbf_g = bass.AP(tensor=wbf[:].tensor,
                                    offset=wbf[:].offset + g,
                                    ap=[list(wbf[:].ap[0]), [6, D], [0, 64]])
                    nc.vector.tensor_tensor(out=wm[:],
                                            in0=msg[:, :, g * 64:(g + 1) * 64],
                                            in1=wbf_g, op=Alu.mult)
                    for c in range(D):
                        nc.tensor.matmul(psum6[:, g, :], lhsT=ident_s[:],
                                         rhs=wm[:, c, :],
                                         start=(c == 0), stop=(c == D - 1))
                # normalize + bias over all 6 graphs at once
                gout = wk_p.tile([128, 6, 64], dt.float32, tag="gout")
                rcol_mid = bass.AP(tensor=rcol[:].tensor,
                                   offset=rcol[:].offset,
                                   ap=[list(rcol[:].ap[0]), [1, 6], [0, 64]])
                nc.vector.tensor_tensor(out=gout[:], in0=psum6[:],
                                        in1=rcol_mid, op=Alu.mult)
                bias_b = bass.AP(tensor=bias[:].tensor,
                                 offset=bias[:].offset,
                                 ap=[list(bias[:].ap[0]), [0, 6], [1, 64]])
                nc.vector.tensor_tensor(out=gout[:], in0=gout[:],
                                        in1=bias_b, op=Alu.add)
                if layer == 1:
                    outrow = rows_p.tile([128, 6, 128], dt.bfloat16,
                                         tag="orow")
                    nc.scalar.activation(outrow[:, :, 0:64], gout[:],
                                         Act.Gelu)
                    nc.sync.dma_start(
                        out=l1out[b // 4][(b % 4) * 128:(b % 4 + 1) * 128, :],
                        in_=outrow[:].rearrange("p g l -> p (g l)"))
                else:
                    lrow6 = rows_p.tile([128, 6, 128], dt.bfloat16,
                                        tag="lrow6")
                    nc.vector.memset(lrow6[:, :, 0:64], 0.0)
                    nc.scalar.activation(lrow6[:, :, 64:128], gout[:],
                                         Act.Gelu)
                    nc.sync.dma_start(
                        out=lstmtab[b * 128:(b + 1) * 128, :, :],
                        in_=lrow6[:])
                if post_block is not None:
                    post_block(b)

        gcn_ctx = contextlib.ExitStack()
        ps_h = gcn_ctx.enter_context(tc.tile_pool(name="ps_h", bufs=2,
                                                  space="PSUM"))
        ps_agg = gcn_ctx.enter_context(tc.tile_pool(name="ps_agg", bufs=2,
                                                    space="PSUM"))

        # ---- layer 1 (AllGather chunks fire as their 4 blocks finish) ----
        def l1_lhsT(g, chn):
            return hfcT[:, g * NPAD + chn * 128: g * NPAD + (chn + 1) * 128]

        def l1_post(b):
            if b % 4 == 3:
                q = b // 4
                nc.gpsimd.collective_compute(
                    "AllGather", Alu.bypass,
                    replica_groups=[[0, 1, 2, 3], [4, 5, 6, 7]],
                    ins=[l1out[q][:].opt()],
                    outs=[agtab[q * 2048:(q + 1) * 2048, :].opt()])

        table_build(1, l1_lhsT)
        sparse_phase(1, post_block=l1_post)
        fc_ctx.close()

        # ---- layer 2 input: gather + 6-way select ----
        l2_ctx = contextlib.ExitStack()
        x2_p = l2_ctx.enter_context(tc.tile_pool(name="x2", bufs=2))
        x2s_p = l2_ctx.enter_context(tc.tile_pool(name="x2s", bufs=1))
        mask_p = l2_ctx.enter_context(tc.tile_pool(name="mask", bufs=2))
        x2sel = []
        for g in range(6):
            xg = x2_p.tile([128, 6, NPAD], dt.bfloat16, tag="xg")
            nc.gpsimd.dma_gather(xg[:], agtab[:], x2idx_s[:, g, :],
                                 NPAD, NPAD, HL, transpose=True,
                                 single_packet=False)
            msk = mask_p.tile([64, 5, NPAD], dt.uint8, tag="msk")
            nc.gpsimd.dma_start(
                out=msk[:],
                in_=bass.AP(tensor=selmask_d.ap().tensor,
                            offset=g * 5 * NPAD,
                            ap=[[0, 64], [NPAD, 5], [1, NPAD]]))
            xs = x2s_p.tile([64, NPAD], dt.bfloat16, name=f"xs{g}")
            nc.vector.tensor_copy(xs[:], xg[0:64, 0, :])
            for k in range(1, 6):
                nc.vector.copy_predicated(
                    xs[:], msk[:, k - 1, :], xg[0:64, k, :])
            x2sel.append(xs)

        # ---- layer 2 ----
        def l2_lhsT(g, chn):
            return x2sel[g][:, chn * 128:(chn + 1) * 128]
        table_build(2, l2_lhsT)
        sparse_phase(2)
        l2_ctx.close()
        gcn_ctx.close()

        # ---- LSTM ----
        lst_ctx = contextlib.ExitStack()
        big = lst_ctx.enter_context(tc.tile_pool(name="big", bufs=1))
        lst_p = lst_ctx.enter_context(tc.tile_pool(name="lst", bufs=2))
        ps_misc = lst_ctx.enter_context(tc.tile_pool(name="ps_misc", bufs=1,
                                                     space="PSUM"))
        ps_z = lst_ctx.enter_context(tc.tile_pool(name="ps_z", bufs=4,
                                                  space="PSUM"))
        lt = lstmtab[:]
        lt_flat = bass.AP(tensor=lt.tensor, offset=lt.offset,
                          ap=[[128, TROWS * 6], [1, 128]])
        XTs = []
        for tg in range(T):
            XT = big.tile([128, 1, 512], dt.bfloat16, name=f"XT{tg}")
            nc.gpsimd.dma_gather(XT[:], lt_flat,
                                 lidx_s[:, tg, :], 512, 500, 128,
                                 transpose=True, single_packet=False)
            XTs.append(XT)
        stacked = big.tile([128, SEQ], dt.bfloat16, name="stacked")
        cT = big.tile([64, SEQ], dt.float32, name="cT")
        h23 = big.tile([64, SEQ], dt.float32, name="h23")
        nc.vector.memset(stacked[0:64, :], 0.0)
        nc.vector.memset(cT[:], 0.0)
        for t in range(T):
            nc.vector.tensor_copy(stacked[64:128, :], XTs[t][64:128, 0, 0:SEQ])
            zs = []
            for g in range(4):
                ps = ps_z.tile([64, SEQ], dt.float32, space="PSUM",
                               tag="pslstm", name=f"z{g}")
                nc.tensor.matmul(ps[:], lhsT=LL_s[:, g * 64:(g + 1) * 64],
                                 rhs=stacked[:], start=True, stop=True)
                zs.append(ps)
            gates = [None] * 4
            for g, fn in ((0, Act.Sigmoid), (1, Act.Sigmoid),
                          (3, Act.Sigmoid), (2, Act.Tanh)):
                gt = lst_p.tile([64, SEQ], dt.float32, tag=f"gate{g}",
                                name=f"gate{g}")
                nc.scalar.activation(gt[:], zs[g][:], fn,
                                     bias=lstmb_s[:, g:g + 1], scale=1.0)
                gates[g] = gt
            ig = lst_p.tile([64, SEQ], dt.float32, tag="ig")
            nc.vector.tensor_mul(ig[:], gates[0][:], gates[2][:])
            nc.vector.tensor_mul(cT[:], gates[1][:], cT[:])
            nc.vector.tensor_add(cT[:], cT[:], ig[:])
            th = lst_p.tile([64, SEQ], dt.float32, tag="th")
            nc.scalar.activation(th[:], cT[:], Act.Tanh)
            if t < T - 1:
                nc.vector.tensor_mul(stacked[0:64, :], gates[3][:], th[:])
            else:
                nc.vector.tensor_mul(h23[:], gates[3][:], th[:])

        # ---- LayerNorm (transposed; stats via ones-matmuls) ----
        ps_mu = ps_misc.tile([1, SEQ], dt.float32, space="PSUM", tag="psmisc",
                             name="ps_mu")
        nc.tensor.matmul(ps_mu[:], lhsT=onescol[0:64, :], rhs=h23[:],
                         start=True, stop=True)
        mu_sb = lst_p.tile([1, SEQ], dt.float32, tag="mu")
        nc.scalar.activation(mu_sb[:], ps_mu[:], Act.Copy, scale=1.0 / 64)
        ps_mub = ps_misc.tile([64, SEQ], dt.float32, space="PSUM",
                              tag="psb500", name="ps_mub")
        nc.tensor.matmul(ps_mub[:], lhsT=onesrow[:], rhs=mu_sb[:],
                         start=True, stop=True)
        dtl = lst_p.tile([64, SEQ], dt.float32, tag="dtl")
        nc.vector.tensor_sub(dtl[:], h23[:], ps_mub[:])
        sq = lst_p.tile([64, SEQ], dt.float32, tag="sq")
        nc.vector.tensor_mul(sq[:], dtl[:], dtl[:])
        ps_var = ps_misc.tile([1, SEQ], dt.float32, space="PSUM", tag="psmisc",
                              name="ps_var")
        nc.tensor.matmul(ps_var[:], lhsT=onescol[0:64, :], rhs=sq[:],
                         start=True, stop=True)
        sd_sb = lst_p.tile([1, SEQ], dt.float32, tag="sd")
        nc.scalar.activation(sd_sb[:], ps_var[:], Act.Sqrt, bias=epsT[:],
                             scale=1.0 / 64)
        rstd = lst_p.tile([1, SEQ], dt.float32, tag="rstd")
        nc.vector.reciprocal(rstd[:], sd_sb[:])
        ps_rb = ps_misc.tile([64, SEQ], dt.float32, space="PSUM",
                             tag="psb500", name="ps_rb")
        nc.tensor.matmul(ps_rb[:], lhsT=onesrow[:], rhs=rstd[:],
                         start=True, stop=True)
        hn = lst_p.tile([64, SEQ], dt.float32, tag="hn")
        nc.vector.tensor_mul(hn[:], dtl[:], ps_rb[:])
        nc.vector.tensor_scalar(out=hn[:], in0=hn[:],
                                scalar1=lng_s[:], scalar2=lnb_s[:],
                                op0=Alu.mult, op1=Alu.add)
        # ---- dense ----
        for q in range(4):
            cs = min(128, SEQ - q * 128)
            psd = ps_misc.tile([128, 12], dt.float32, space="PSUM",
                               tag="psmisc", name="psd")
            nc.tensor.matmul(psd[0:cs, :],
                             lhsT=hn[:, q * 128:q * 128 + cs],
                             rhs=dW_s[:], start=True, stop=True)
            ot = lst_p.tile([128, 12], dt.float32, tag="ot")
            nc.vector.tensor_add(ot[0:cs, :], psd[0:cs, :], dbR[0:cs, :])
            nc.sync.dma_start(out=out_d[q * 128:q * 128 + cs, :],
                              in_=ot[0:cs, :])
        lst_ctx.close()
        ctx.close()

    nc.compile()
    return nc


# ---------------------------------------------------------------------------
# Runner
# ---------------------------------------------------------------------------

_CACHE = {}


def _get_program_and_maps(inputs):
    st = prep_structure(inputs["edge_src"], inputs["edge_dst"])
    w = prep_weights(inputs)
    meta = dict(dmax=st["dmax"], coloff=st["coloff"], ncol=st["ncol"])
    key = ("prog", tuple(st["dmax"]))
    if key not in _CACHE:
        _CACHE[key] = build_program(meta)
    nc = _CACHE[key]

    shared = dict(eidx=st["eidx"], lstmidx=st["lstmidx"], fcW=w["fcW"],
                  fcb=w["fcb"], W1c=w["W1c"], W2c=w["W2c"],
                  b1=w["b1"], b2=w["b2"], LL=w["LL"], lstmb=w["lstmb"],
                  lng=w["lng"], lnb=w["lnb"], dW=w["dW"], db=w["db"],
                  dumrow=w["dumrow"], z384=w["z384"], ident=w["ident"])
    in_maps = []
    for core in range(NCORES):
        pc = prep_core(st, core)
        m = dict(shared)
        m["xT"] = prep_xT(inputs, st, core)
        m["x2idx"] = pc["x2idx"]
        m["selmask"] = pc["selmask"]
        in_maps.append(m)
    return nc, in_maps


def run_on_hw(inputs, trace=False):
    _ensure_hook()
    from concourse.bass_utils import run_bass_kernel_spmd
    nc, in_maps = _get_program_and_maps(inputs)
    res = run_bass_kernel_spmd(nc, in_maps, list(range(NCORES)), trace=trace)
    out_full = np.zeros((B, N, PL, 1), np.float32)
    for core in range(NCORES):
        bb, c = core // 4, core % 4
        out_full[bb, 500 * c:500 * (c + 1), :, 0] = res.results[core]["out"]
    return out_full, res


def kernel(x, edge_src, edge_dst, fc_W, fc_b,
           g1_W, g1_b, g1_asrc, g1_adst,
           g2_W, g2_b, g2_asrc, g2_adst,
           lstm_Wih, lstm_Whh, lstm_bih, lstm_bhh,
           ln_g, ln_b, dense_W, dense_b):
    inputs = dict(x=x, edge_src=edge_src, edge_dst=edge_dst, fc_W=fc_W,
                  fc_b=fc_b, g1_W=g1_W, g1_b=g1_b, g1_asrc=g1_asrc,
                  g1_adst=g1_adst, g2_W=g2_W, g2_b=g2_b, g2_asrc=g2_asrc,
                  g2_adst=g2_adst, lstm_Wih=lstm_Wih, lstm_Whh=lstm_Whh,
                  lstm_bih=lstm_bih, lstm_bhh=lstm_bhh, ln_g=ln_g, ln_b=ln_b,
                  dense_W=dense_W, dense_b=dense_b)
    out, _ = run_on_hw(inputs, trace=False)
    return out


# revision 27
# speedup vs baseline: 816.3357x; 1.0205x over previous
"""GCN(GAT)x2 + LSTM + LN + dense on 8 Trainium2 NeuronCores (Bass/Tile).

V2: data-parallel over B*T=48 graphs (6 per core). All 6 graphs of a core
pack into ONE h-table (1KB rows: 6x64 h lanes + 6 es lanes), so the
per-edge slot gather costs one Q7 descriptor stream per layer instead of
three. es/ed come from the same matmul as h (rhs = [W | W@a_s | W@a_d]).
Softmax drops the max-shift (est bounded ~|6.5|, exp safe in f32). The
per-edge weighted slot-sum runs on the Tensor engine as identity-matmul
PSUM accumulation; DVE only does the est chain + one blast-multiply per
graph. LSTM stage unchanged (core-local via the scramble block structure).
"""

import os
import sys
import numpy as np
import ml_dtypes

BF16 = ml_dtypes.bfloat16

N = 2000
NPAD = 2048
T = 24
B = 2
NX = 8
H = 64
E = 32000
PL = 12
NB = 16          # dst blocks of 128 pi-positions
DUM = 2048       # dummy table row index
TROWS = 2049     # rows in the 6-graph table
LANES = 512      # table row width (bf16): 6*64 h | 6 es | pad
ESL = 384        # es lane base
HL = 768         # l1out row width (6 x 128-lane chunks, h at chunk base)
NEG = -1e9
LN_EPS = 1e-5
NCORES = 8
SEQ = 500        # LSTM sequences per core


def _ensure_hook():
    """Register the NTFF profile hook if the boot didn't (enables traces)."""
    try:
        try:
            import antenv.axon_hooks  # noqa: F401
        except ImportError:
            import types
            import antenv
            mod = types.ModuleType("antenv.axon_hooks")
            mod._hook = None

            def set_axon_ntff_profile_hook(h, _m=mod):
                _m._hook = h

            def get_axon_ntff_profile_hook(_m=mod):
                return _m._hook

            mod.set_axon_ntff_profile_hook = set_axon_ntff_profile_hook
            mod.get_axon_ntff_profile_hook = get_axon_ntff_profile_hook
            sys.modules["antenv.axon_hooks"] = mod
            antenv.axon_hooks = mod
        from antenv.axon_hooks import (get_axon_ntff_profile_hook,
                                       set_axon_ntff_profile_hook)
        if get_axon_ntff_profile_hook() is None:
            from trn_agent_boot.trn_boot import _ntff_profile_via_ctypes
            so = "/opt/axon/libaxon_pjrt.so"
            if os.path.exists(so):
                set_axon_ntff_profile_hook(_ntff_profile_via_ctypes(so))
    except Exception:
        pass


# ---------------------------------------------------------------------------
# Host-side prep
# ---------------------------------------------------------------------------

def fmt_idx(flat):
    flat = np.asarray(flat)
    assert len(flat) % 128 == 0
    fmt = flat.reshape(-1, 16).T.astype(np.int16)
    return np.tile(fmt, (8, 1))


def prep_structure(edge_src, edge_dst):
    src = np.asarray(edge_src, np.int64)
    dst = np.asarray(edge_dst, np.int64)
    deg = np.bincount(dst, minlength=N)
    pi = np.argsort(deg, kind="stable")
    pi_inv = np.empty(N, np.int64)
    pi_inv[pi] = np.arange(N)

    dpos = pi_inv[dst]
    order = np.argsort(dpos, kind="stable")
    src_s = src[order]
    dpos_s = dpos[order]
    cnt = np.bincount(dpos_s, minlength=NPAD)
    dmax = [int(max(1, cnt[b * 128:(b + 1) * 128].max())) for b in range(NB)]
    coloff = np.concatenate([[0], np.cumsum(dmax)]).astype(np.int64)
    ncol = int(coloff[-1])

    starts = np.concatenate([[0], np.cumsum(cnt)]).astype(np.int64)
    slot = np.full((ncol, 128), DUM, np.int64)
    b_of = dpos_s // 128
    p_of = dpos_s % 128
    jw = np.arange(E) - starts[dpos_s]
    slot[coloff[b_of] + jw, p_of] = pi_inv[src_s]
    eidx = fmt_idx(slot.reshape(-1))

    # LSTM gather rows: (t, local-seq nl) -> lstmtab row node_pi*6 + lg
    per_t = []
    for t in range(T):
        nl = np.arange(SEQ)
        flat_i = nl * T + t
        lg = flat_i // N
        lrow = pi_inv[flat_i % N] * 6 + lg
        lrow = np.concatenate([lrow, np.zeros(12, np.int64)])
        per_t.append(fmt_idx(lrow))  # [128, 32]
    lstmidx = np.ascontiguousarray(np.stack(per_t, axis=1))  # [128, 24, 32]

    return dict(pi=pi, pi_inv=pi_inv, dmax=dmax, coloff=coloff, ncol=ncol,
                eidx=eidx, lstmidx=lstmidx)


def prep_core(st, core):
    pi, pi_inv = st["pi"], st["pi_inv"]
    c = core % 4
    x2rows = np.full((6, NPAD), DUM, np.int64)
    sel = np.zeros((6, NPAD), np.int64)
    ppos = np.arange(N)
    for lg in range(6):
        t2 = 6 * c + lg
        flat_i = pi[ppos] * T + t2
        tp, npr = flat_i // N, flat_i % N
        # agtab row: [chunk(4 blocks), src-core, row-in-chunk]
        q = pi_inv[npr] // 512
        x2rows[lg, :N] = q * 2048 + (tp // 6) * 512 + pi_inv[npr] % 512
        sel[lg, :N] = tp % 6
    x2rows[:, N:] = 0
    x2idx = np.stack([fmt_idx(x2rows[lg]) for lg in range(6)], axis=1)
    # 5 predicated-select masks per graph (k=1..5); k=0 is the base copy.
    selmask = np.zeros((6, 5, NPAD), np.uint8)
    for lg in range(6):
        for k in range(1, 6):
            selmask[lg, k - 1] = (sel[lg] == k).astype(np.uint8)
    return dict(x2idx=np.ascontiguousarray(x2idx.astype(np.int16)),
                selmask=selmask)


def prep_weights(p):
    f32 = np.float32
    w = {}
    w["fcW"] = np.asarray(p["fc_W"], f32).astype(BF16)
    w["fcb"] = np.asarray(p["fc_b"], f32).reshape(64, 1)

    def wc(tag):
        W = np.asarray(p[f"{tag}_W"], f32)
        a_s = np.asarray(p[f"{tag}_asrc"], f32).reshape(64, 1)
        a_d = np.asarray(p[f"{tag}_adst"], f32).reshape(64, 1)
        return np.hstack([W, W @ a_s, W @ a_d]).astype(BF16)  # [64, 66]

    w["W1c"] = wc("g1")
    w["W2c"] = wc("g2")
    w["b1"] = np.asarray(p["g1_b"], f32).reshape(1, 64)
    w["b2"] = np.asarray(p["g2_b"], f32).reshape(1, 64)
    Wih = np.asarray(p["lstm_Wih"], f32)
    Whh = np.asarray(p["lstm_Whh"], f32)
    w["LL"] = np.vstack([Whh.T, Wih.T]).astype(BF16)  # K rows: 0:64=h, 64:128=x
    bihh = np.asarray(p["lstm_bih"], f32) + np.asarray(p["lstm_bhh"], f32)
    w["lstmb"] = np.ascontiguousarray(bihh.reshape(4, 64).T)  # [64,4] i,f,g,o
    w["lng"] = np.asarray(p["ln_g"], f32).reshape(64, 1)
    w["lnb"] = np.asarray(p["ln_b"], f32).reshape(64, 1)
    w["dW"] = np.asarray(p["dense_W"], f32)
    w["db"] = np.asarray(p["dense_b"], f32).reshape(1, 12)
    dum = np.zeros((1, LANES), f32)
    dum[0, ESL:ESL + 6] = NEG
    w["dumrow"] = dum.astype(BF16)
    w["z384"] = np.zeros((1, HL), BF16)
    w["ident"] = np.eye(128, dtype=BF16)
    return w


def prep_xT(p, st, core):
    x = np.asarray(p["x"], np.float32)
    b, c = core // 4, core % 4
    pi = st["pi"]
    xT = np.zeros((NX, 6 * NPAD), np.float32)
    for lg in range(6):
        xT[:, lg * NPAD: lg * NPAD + N] = x[b, pi, 6 * c + lg, :].T
    return np.ascontiguousarray(xT.astype(BF16))


# ---------------------------------------------------------------------------
# Bass program
# ---------------------------------------------------------------------------

def build_program(meta):
    import concourse.bass as bass
    import concourse.mybir as mybir
    import concourse.tile as tile
    from concourse import bacc

    dt = mybir.dt
    Alu = mybir.AluOpType
    Act = mybir.ActivationFunctionType
    AX = mybir.AxisListType
    dmax, coloff, ncol = meta["dmax"], meta["coloff"], meta["ncol"]

    nc = bacc.Bacc("TRN2", target_bir_lowering=False, debug=False,
                   num_devices=NCORES)

    def din(name, shape, dty):
        return nc.dram_tensor(name, list(shape), dty, kind="ExternalInput")

    xT_d = din("xT", (NX, 6 * NPAD), dt.bfloat16)
    eidx_d = din("eidx", (128, ncol * 8), dt.int16)
    x2idx_d = din("x2idx", (128, 6, 128), dt.int16)
    lstmidx_d = din("lstmidx", (128, T, 32), dt.int16)
    selmask_d = din("selmask", (6, 5, NPAD), dt.uint8)
    fcW_d = din("fcW", (NX, 64), dt.bfloat16)
    fcb_d = din("fcb", (64, 1), dt.float32)
    W1c_d = din("W1c", (64, 66), dt.bfloat16)
    W2c_d = din("W2c", (64, 66), dt.bfloat16)
    b1_d = din("b1", (1, 64), dt.float32)
    b2_d = din("b2", (1, 64), dt.float32)
    LL_d = din("LL", (128, 256), dt.bfloat16)
    lstmb_d = din("lstmb", (64, 4), dt.float32)
    lng_d = din("lng", (64, 1), dt.float32)
    lnb_d = din("lnb", (64, 1), dt.float32)
    dW_d = din("dW", (64, 12), dt.float32)
    db_d = din("db", (1, 12), dt.float32)
    dum_d = din("dumrow", (1, LANES), dt.bfloat16)
    z384_d = din("z384", (1, HL), dt.bfloat16)
    ident_d = din("ident", (128, 128), dt.bfloat16)
    out_d = nc.dram_tensor("out", [SEQ, 12], dt.float32, kind="ExternalOutput")

    def bcast_ap(dram_t, parts, inner):
        """Partition-broadcast read AP of a DRAM region."""
        ap = dram_t if isinstance(dram_t, bass.AP) else dram_t.ap()
        return bass.AP(tensor=ap.tensor, offset=ap.offset,
                       ap=[[0, parts]] + [list(x) for x in ap.ap[1:]])

    def blast(ap_, n):
        """Append a step-0 dim of size n to an AP (free-dim broadcast)."""
        return bass.AP(tensor=ap_.tensor, offset=ap_.offset,
                       ap=list(ap_.ap) + [[0, n]])

    def midblast(ap_, n):
        """Insert a step-0 dim of size n before the last dim of an AP."""
        return bass.AP(tensor=ap_.tensor, offset=ap_.offset,
                       ap=list(ap_.ap[:-1]) + [[0, n], list(ap_.ap[-1])])

    with tile.TileContext(nc) as tc:
        import contextlib
        ctx = contextlib.ExitStack()
        consts = ctx.enter_context(tc.tile_pool(name="consts", bufs=1))
        dramp = ctx.enter_context(tc.tile_pool(name="dramp", bufs=1,
                                               space="DRAM"))
        rows_p = ctx.enter_context(tc.tile_pool(name="rows", bufs=3))
        msg_p = ctx.enter_context(tc.tile_pool(name="msg", bufs=2))
        wk_p = ctx.enter_context(tc.tile_pool(name="wk", bufs=4))
        wm_p = ctx.enter_context(tc.tile_pool(name="wm", bufs=3))
        edt_p = ctx.enter_context(tc.tile_pool(name="edt", bufs=1))

        # ---- constants into SBUF ----
        def load(name, shape, dty, src_ap):
            t = consts.tile(shape, dty, name=name)
            nc.sync.dma_start(out=t[:], in_=src_ap)
            return t

        eidx_s = load("eidx_s", [128, ncol * 8], dt.int16, eidx_d[:])
        x2idx_s = load("x2idx_s", [128, 6, 128], dt.int16, x2idx_d[:])
        lidx_s = load("lidx_s", [128, T, 32], dt.int16, lstmidx_d[:])
        fcW_s = load("fcW_s", [NX, 64], dt.bfloat16, fcW_d[:])
        fcb_s = load("fcb_s", [64, 1], dt.float32, fcb_d[:])
        W1c_s = load("W1c_s", [64, 66], dt.bfloat16, W1c_d[:])
        W2c_s = load("W2c_s", [64, 66], dt.bfloat16, W2c_d[:])
        LL_s = load("LL_s", [128, 256], dt.bfloat16, LL_d[:])
        dW_s = load("dW_s", [64, 12], dt.float32, dW_d[:])
        lstmb_s = load("lstmb_s", [64, 4], dt.float32, lstmb_d[:])
        lng_s = load("lng_s", [64, 1], dt.float32, lng_d[:])
        lnb_s = load("lnb_s", [64, 1], dt.float32, lnb_d[:])
        ident_s = load("ident_s", [128, 128], dt.bfloat16, ident_d[:])
        bR = {}
        for nm, d_ in (("b1", b1_d), ("b2", b2_d)):
            t = consts.tile([128, 64], dt.float32, name=nm + "R")
            nc.gpsimd.dma_start(out=t[:], in_=bcast_ap(d_, 128, 64))
            bR[nm] = t
        dbR = consts.tile([128, 12], dt.float32, name="dbR")
        nc.gpsimd.dma_start(out=dbR[:], in_=bcast_ap(db_d, 128, 12))
        onesrow = consts.tile([1, 64], dt.float32, name="onesrow")
        nc.vector.memset(onesrow[:], 1.0)
        onescol = consts.tile([128, 1], dt.float32, name="onescol")
        nc.vector.memset(onescol[:], 1.0)
        epsT = consts.tile([1, 1], dt.float32, name="epsT")
        nc.vector.memset(epsT[:], LN_EPS)

        # ---- DRAM tables ----
        htab = {1: dramp.tile([TROWS, LANES], dt.bfloat16, name="htab1"),
                2: dramp.tile([TROWS, LANES], dt.bfloat16, name="htab2")}
        # l1out in 4 chunks of 4 dst-blocks so each AllGather chunk can
        # launch as soon as its blocks are done (slice-level dep via tiles).
        l1out = [dramp.tile([512, HL], dt.bfloat16, name=f"l1out{q}")
                 for q in range(4)]
        agtab = dramp.tile([4 * 2048, HL], dt.bfloat16, name="agtab")
        lstmtab = dramp.tile([TROWS, 6, 128], dt.bfloat16, name="lstmtab")

        edt_all = {1: edt_p.tile([128, NB, 6], dt.float32, name="edt1"),
                   2: edt_p.tile([128, NB, 6], dt.float32, name="edt2")}

        # ---- FC: hfcT bf16 = fcW.T @ xT + fcb, split in 4 chunk-group
        # tiles (chn-group q holds cols g*512 + (chn%4)*128) so the L1
        # table build can start on group 0 while FC fills groups 1-3.
        fc_ctx = contextlib.ExitStack()
        hfc_p = fc_ctx.enter_context(tc.tile_pool(name="hfc", bufs=1))
        hfcQ = [hfc_p.tile([64, 6 * 512], dt.bfloat16, name=f"hfcT{q}")
                for q in range(4)]
        with tc.tile_pool(name="xtp", bufs=1) as xt_p, \
             tc.tile_pool(name="ps_fc", bufs=2, space="PSUM") as ps_fc:
            xT_s = xt_p.tile([NX, 6 * NPAD], dt.bfloat16, name="xT_s")
            nc.sync.dma_start(out=xT_s[:], in_=xT_d[:])
            for q in range(4):
                for g in range(6):
                    sl = slice(g * NPAD + q * 512, g * NPAD + (q + 1) * 512)
                    ps = ps_fc.tile([64, 512], dt.float32, space="PSUM",
                                    tag="psfc")
                    nc.tensor.matmul(ps[:], lhsT=fcW_s[:], rhs=xT_s[:, sl],
                                     start=True, stop=True)
                    nc.vector.tensor_scalar_add(
                        hfcQ[q][:, g * 512:(g + 1) * 512], ps[:], fcb_s[:])

        def table_build(layer, lhsT_of):
            """Build the 6-graph h-table + edt for one layer."""
            Wc = W1c_s if layer == 1 else W2c_s
            edt = edt_all[layer]
            for chn in range(NB):
                rowt = rows_p.tile([128, LANES], dt.bfloat16, tag="rowt")
                for g in range(6):
                    ps = ps_h.tile([128, 66], dt.float32, space="PSUM",
                                   tag="psh")
                    nc.tensor.matmul(ps[:], lhsT=lhsT_of(g, chn), rhs=Wc[:],
                                     start=True, stop=True)
                    nc.scalar.activation(rowt[:, g * 64:(g + 1) * 64],
                                         ps[:, 0:64], Act.Copy)
                    nc.vector.tensor_copy(rowt[:, ESL + g:ESL + g + 1],
                                          ps[:, 64:65])
                    nc.vector.tensor_copy(edt[:, chn, g:g + 1],
                                          ps[:, 65:66])
                nc.sync.dma_start(
                    out=htab[layer][chn * 128:(chn + 1) * 128, 0:ESL + 6],
                    in_=rowt[:, 0:ESL + 6])
            nc.sync.dma_start(out=htab[layer][DUM:DUM + 1, :], in_=dum_d[:])

        def sparse_phase(layer, post_block=None):
            """Per-block gather + GAT softmax/aggregate, all 6 graphs."""
            bias = bR["b1"] if layer == 1 else bR["b2"]
            edt = edt_all[layer]
            for b in range(NB):
                D = dmax[b]
                msg = msg_p.tile([128, D, LANES], dt.bfloat16, tag="msg")
                nc.gpsimd.dma_gather(
                    msg[:], htab[layer],
                    eidx_s[:, 8 * int(coloff[b]): 8 * int(coloff[b] + D)],
                    128 * D, 128 * D, LANES, single_packet=False)
                # est [128, D, 6] = es(src) + ed(dst)
                est = wk_p.tile([128, D, 6], dt.float32, tag="est")
                edt_b = bass.AP(tensor=edt[:].tensor,
                                offset=edt[:].offset + b * 6,
                                ap=[list(edt[:].ap[0]), [0, D], [1, 6]])
                nc.vector.tensor_tensor(out=est[:],
                                        in0=msg[:, :, ESL:ESL + 6],
                                        in1=edt_b, op=Alu.add)
                estL = wk_p.tile([128, D * 6], dt.float32, tag="estL")
                ef = est[:].rearrange("p d g -> p (d g)")
                nc.vector.scalar_tensor_tensor(out=estL[:], in0=ef,
                                               scalar=0.2, in1=ef,
                                               op0=Alu.mult, op1=Alu.max)
                wbf = wk_p.tile([128, D, 6], dt.bfloat16, tag="wbf")
                nc.scalar.activation(
                    wbf[:].rearrange("p d g -> p (d g)"), estL[:], Act.Exp)
                den = wk_p.tile([128, 6], dt.float32, tag="den")
                nc.vector.tensor_reduce(den[:],
                                        wbf[:].rearrange("p d g -> p g d"),
                                        axis=AX.X, op=Alu.add)
                rcol = wk_p.tile([128, 6], dt.float32, tag="rcol")
                nc.vector.reciprocal(rcol[:], den[:])
                psum6 = ps_agg.tile([128, 6, 64], dt.float32, space="PSUM",
                                    tag="psagg")
                for g in range(6):
                    wm = wm_p.tile([128, D, 64], dt.bfloat16, tag="wm")
                    wbf_g = bass.AP(tensor=wbf[:].tensor,
                                    offset=wbf[:].offset + g,
                                    ap=[list(wbf[:].ap[0]), [6, D], [0, 64]])
                    nc.vector.tensor_tensor(out=wm[:],
                                            in0=msg[:, :, g * 64:(g + 1) * 64],
                                            in1=wbf_g, op=Alu.mult)
                    for c in range(D):
                        nc.tensor.matmul(psum6[:, g, :], lhsT=ident_s[:],
                                         rhs=wm[:, c, :],
                                         start=(c == 0), stop=(c == D - 1))
                # normalize + bias over all 6 graphs at once
                gout = wk_p.tile([128, 6, 64], dt.float32, tag="gout")
                rcol_mid = bass.AP(tensor=rcol[:].tensor,
                                   offset=rcol[:].offset,
                                   ap=[list(rcol[:].ap[0]), [1, 6], [0, 64]])
                nc.vector.tensor_tensor(out=gout[:], in0=psum6[:],
                                        in1=rcol_mid, op=Alu.mult)
                bias_b = bass.AP(tensor=bias[:].tensor,
                                 offset=bias[:].offset,
                                 ap=[list(bias[:].ap[0]), [0, 6], [1, 64]])
                nc.vector.tensor_tensor(out=gout[:], in0=gout[:],
                                        in1=bias_b, op=Alu.add)
                if layer == 1:
                    outrow = rows_p.tile([128, 6, 128], dt.bfloat16,
                                         tag="orow")
                    nc.scalar.activation(outrow[:, :, 0:64], gout[:],
                                         Act.Gelu)
                    nc.sync.dma_start(
                        out=l1out[b // 4][(b % 4) * 128:(b % 4 + 1) * 128, :],
                        in_=outrow[:].rearrange("p g l -> p (g l)"))
                else:
                    lrow6 = rows_p.tile([128, 6, 128], dt.bfloat16,
                                        tag="lrow6")
                    nc.vector.memset(lrow6[:, :, 0:64], 0.0)
                    nc.scalar.activation(lrow6[:, :, 64:128], gout[:],
                                         Act.Gelu)
                    nc.sync.dma_start(
                        out=lstmtab[b * 128:(b + 1) * 128, :, :],
                        in_=lrow6[:])
                if post_block is not None:
                    post_block(b)

        gcn_ctx = contextlib.ExitStack()
        ps_h = gcn_ctx.enter_context(tc.tile_pool(name="ps_h", bufs=2,
                                                  space="PSUM"))
        ps_agg = gcn_ctx.enter_context(tc.tile_pool(name="ps_agg", bufs=2,
                                                    space="PSUM"))

        # ---- layer 1 (AllGather chunks fire as their 4 blocks finish) ----
        def l1_lhsT(g, chn):
            base = g * 512 + (chn % 4) * 128
            return hfcQ[chn // 4][:, base: base + 128]

        def l1_post(b):
            if b % 4 == 3:
                q = b // 4
                nc.gpsimd.collective_compute(
                    "AllGather", Alu.bypass,
                    replica_groups=[[0, 1, 2, 3], [4, 5, 6, 7]],
                    ins=[l1out[q][:].opt()],
                    outs=[agtab[q * 2048:(q + 1) * 2048, :].opt()])

        table_build(1, l1_lhsT)
        sparse_phase(1, post_block=l1_post)
        fc_ctx.close()

        # ---- layer 2 input: gather + 6-way select ----
        l2_ctx = contextlib.ExitStack()
        x2_p = l2_ctx.enter_context(tc.tile_pool(name="x2", bufs=2))
        x2s_p = l2_ctx.enter_context(tc.tile_pool(name="x2s", bufs=1))
        mask_p = l2_ctx.enter_context(tc.tile_pool(name="mask", bufs=2))
        x2sel = []
        for g in range(6):
            xg = x2_p.tile([128, 6, NPAD], dt.bfloat16, tag="xg")
            nc.gpsimd.dma_gather(xg[:], agtab[:], x2idx_s[:, g, :],
                                 NPAD, NPAD, HL, transpose=True,
                                 single_packet=False)
            msk = mask_p.tile([64, 5, NPAD], dt.uint8, tag="msk")
            nc.gpsimd.dma_start(
                out=msk[:],
                in_=bass.AP(tensor=selmask_d.ap().tensor,
                            offset=g * 5 * NPAD,
                            ap=[[0, 64], [NPAD, 5], [1, NPAD]]))
            xs = x2s_p.tile([64, NPAD], dt.bfloat16, name=f"xs{g}")
            nc.vector.tensor_copy(xs[:], xg[0:64, 0, :])
            for k in range(1, 6):
                nc.vector.copy_predicated(
                    xs[:], msk[:, k - 1, :], xg[0:64, k, :])
            x2sel.append(xs)

        # ---- layer 2 ----
        def l2_lhsT(g, chn):
            return x2sel[g][:, chn * 128:(chn + 1) * 128]
        table_build(2, l2_lhsT)
        sparse_phase(2)
        l2_ctx.close()
        gcn_ctx.close()

        # ---- LSTM ----
        lst_ctx = contextlib.ExitStack()
        big = lst_ctx.enter_context(tc.tile_pool(name="big", bufs=1))
        lst_p = lst_ctx.enter_context(tc.tile_pool(name="lst", bufs=2))
        ps_misc = lst_ctx.enter_context(tc.tile_pool(name="ps_misc", bufs=1,
                                                     space="PSUM"))
        ps_z = lst_ctx.enter_context(tc.tile_pool(name="ps_z", bufs=4,
                                                  space="PSUM"))
        lt = lstmtab[:]
        lt_flat = bass.AP(tensor=lt.tensor, offset=lt.offset,
                          ap=[[128, TROWS * 6], [1, 128]])
        XTs = []
        for tg in range(T):
            XT = big.tile([128, 1, 512], dt.bfloat16, name=f"XT{tg}")
            nc.gpsimd.dma_gather(XT[:], lt_flat,
                                 lidx_s[:, tg, :], 512, 500, 128,
                                 transpose=True, single_packet=False)
            XTs.append(XT)
        stacked = big.tile([128, SEQ], dt.bfloat16, name="stacked")
        cT = big.tile([64, SEQ], dt.float32, name="cT")
        h23 = big.tile([64, SEQ], dt.float32, name="h23")
        nc.vector.memset(stacked[0:64, :], 0.0)
        nc.vector.memset(cT[:], 0.0)
        for t in range(T):
            nc.vector.tensor_copy(stacked[64:128, :], XTs[t][64:128, 0, 0:SEQ])
            zs = []
            for g in range(4):
                ps = ps_z.tile([64, SEQ], dt.float32, space="PSUM",
                               tag="pslstm", name=f"z{g}")
                nc.tensor.matmul(ps[:], lhsT=LL_s[:, g * 64:(g + 1) * 64],
                                 rhs=stacked[:], start=True, stop=True)
                zs.append(ps)
            gates = [None] * 4
            for g, fn in ((0, Act.Sigmoid), (1, Act.Sigmoid),
                          (3, Act.Sigmoid), (2, Act.Tanh)):
                gt = lst_p.tile([64, SEQ], dt.float32, tag=f"gate{g}",
                                name=f"gate{g}")
                nc.scalar.activation(gt[:], zs[g][:], fn,
                                     bias=lstmb_s[:, g:g + 1], scale=1.0)
                gates[g] = gt
            ig = lst_p.tile([64, SEQ], dt.float32, tag="ig")
            nc.vector.tensor_mul(ig[:], gates[0][:], gates[2][:])
            nc.vector.tensor_mul(cT[:], gates[1][:], cT[:])
            nc.vector.tensor_add(cT[:], cT[:], ig[:])
            th = lst_p.tile([64, SEQ], dt.float32, tag="th")
            nc.scalar.activation(th[:], cT[:], Act.Tanh)
            if t < T - 1:
                nc.vector.tensor_mul(stacked[0:64, :], gates[3][:], th[:])
            else:
                nc.vector.tensor_mul(h23[:], gates[3][:], th[:])

        # ---- LayerNorm (transposed; stats via ones-matmuls) ----
        ps_mu = ps_misc.tile([1, SEQ], dt.float32, space="PSUM", tag="psmisc",
                             name="ps_mu")
        nc.tensor.matmul(ps_mu[:], lhsT=onescol[0:64, :], rhs=h23[:],
                         start=True, stop=True)
        mu_sb = lst_p.tile([1, SEQ], dt.float32, tag="mu")
        nc.scalar.activation(mu_sb[:], ps_mu[:], Act.Copy, scale=1.0 / 64)
        ps_mub = ps_misc.tile([64, SEQ], dt.float32, space="PSUM",
                              tag="psb500", name="ps_mub")
        nc.tensor.matmul(ps_mub[:], lhsT=onesrow[:], rhs=mu_sb[:],
                         start=True, stop=True)
        dtl = lst_p.tile([64, SEQ], dt.float32, tag="dtl")
        nc.vector.tensor_sub(dtl[:], h23[:], ps_mub[:])
        sq = lst_p.tile([64, SEQ], dt.float32, tag="sq")
        nc.vector.tensor_mul(sq[:], dtl[:], dtl[:])
        ps_var = ps_misc.tile([1, SEQ], dt.float32, space="PSUM", tag="psmisc",
                              name="ps_var")
        nc.tensor.matmul(ps_var[:], lhsT=onescol[0:64, :], rhs=sq[:],
                         start=True, stop=True)
        sd_sb = lst_p.tile([1, SEQ], dt.float32, tag="sd")
        nc.scalar.activation(sd_sb[:], ps_var[:], Act.Sqrt, bias=epsT[:],
                             scale=1.0 / 64)
        rstd = lst_p.tile([1, SEQ], dt.float32, tag="rstd")
        nc.vector.reciprocal(rstd[:], sd_sb[:])
        ps_rb = ps_misc.tile([64, SEQ], dt.float32, space="PSUM",
                             tag="psb500", name="ps_rb")
        nc.tensor.matmul(ps_rb[:], lhsT=onesrow[:], rhs=rstd[:],
                         start=True, stop=True)
        hn = lst_p.tile([64, SEQ], dt.float32, tag="hn")
        nc.vector.tensor_mul(hn[:], dtl[:], ps_rb[:])
        nc.vector.tensor_scalar(out=hn[:], in0=hn[:],
                                scalar1=lng_s[:], scalar2=lnb_s[:],
                                op0=Alu.mult, op1=Alu.add)
        # ---- dense ----
        for q in range(4):
            cs = min(128, SEQ - q * 128)
            psd = ps_misc.tile([128, 12], dt.float32, space="PSUM",
                               tag="psmisc", name="psd")
            nc.tensor.matmul(psd[0:cs, :],
                             lhsT=hn[:, q * 128:q * 128 + cs],
                             rhs=dW_s[:], start=True, stop=True)
            ot = lst_p.tile([128, 12], dt.float32, tag="ot")
            nc.vector.tensor_add(ot[0:cs, :], psd[0:cs, :], dbR[0:cs, :])
            nc.sync.dma_start(out=out_d[q * 128:q * 128 + cs, :],
                              in_=ot[0:cs, :])
        lst_ctx.close()
        ctx.close()

    nc.compile()
    return nc


# ---------------------------------------------------------------------------
# Runner
# ---------------------------------------------------------------------------

_CACHE = {}


def _get_program_and_maps(inputs):
    st = prep_structure(inputs["edge_src"], inputs["edge_dst"])
    w = prep_weights(inputs)
    meta = dict(dmax=st["dmax"], coloff=st["coloff"], ncol=st["ncol"])
    key = ("prog", tuple(st["dmax"]))
    if key not in _CACHE:
        _CACHE[key] = build_program(meta)
    nc = _CACHE[key]

    shared = dict(eidx=st["eidx"], lstmidx=st["lstmidx"], fcW=w["fcW"],
                  fcb=w["fcb"], W1c=w["W1c"], W2c=w["W2c"],
                  b1=w["b1"], b2=w["b2"], LL=w["LL"], lstmb=w["lstmb"],
                  lng=w["lng"], lnb=w["lnb"], dW=w["dW"], db=w["db"],
                  dumrow=w["dumrow"], z384=w["z384"], ident=w["ident"])
    in_maps = []
    for core in range(NCORES):
        pc = prep_core(st, core)
        m = dict(shared)
        m["xT"] = prep_xT(inputs, st, core)
        m["x2idx"] = pc["x2idx"]
        m["selmask"] = pc["selmask"]
        in_maps.append(m)
    return nc, in_maps


def run_on_hw(inputs, trace=False):
    _ensure_hook()
    from concourse.bass_utils import run_bass_kernel_spmd
    nc, in_maps = _get_program_and_maps(inputs)
    res = run_bass_kernel_spmd(nc, in_maps, list(range(NCORES)), trace=trace)
    out_full = np.zeros((B, N, PL, 1), np.float32)
    for core in range(NCORES):
        bb, c = core // 4, core % 4
        out_full[bb, 500 * c:500 * (c + 1), :, 0] = res.results[core]["out"]
    return out_full, res


def kernel(x, edge_src, edge_dst, fc_W, fc_b,
           g1_W, g1_b, g1_asrc, g1_adst,
           g2_W, g2_b, g2_asrc, g2_adst,
           lstm_Wih, lstm_Whh, lstm_bih, lstm_bhh,
           ln_g, ln_b, dense_W, dense_b):
    inputs = dict(x=x, edge_src=edge_src, edge_dst=edge_dst, fc_W=fc_W,
                  fc_b=fc_b, g1_W=g1_W, g1_b=g1_b, g1_asrc=g1_asrc,
                  g1_adst=g1_adst, g2_W=g2_W, g2_b=g2_b, g2_asrc=g2_asrc,
                  g2_adst=g2_adst, lstm_Wih=lstm_Wih, lstm_Whh=lstm_Whh,
                  lstm_bih=lstm_bih, lstm_bhh=lstm_bhh, ln_g=ln_g, ln_b=ln_b,
                  dense_W=dense_W, dense_b=dense_b)
    out, _ = run_on_hw(inputs, trace=False)
    return out


# revision 29
# speedup vs baseline: 856.9597x; 1.0498x over previous
"""GCN(GAT)x2 + LSTM + LN + dense on 8 Trainium2 NeuronCores (Bass/Tile).

V2: data-parallel over B*T=48 graphs (6 per core). All 6 graphs of a core
pack into ONE h-table (1KB rows: 6x64 h lanes + 6 es lanes), so the
per-edge slot gather costs one Q7 descriptor stream per layer instead of
three. es/ed come from the same matmul as h (rhs = [W | W@a_s | W@a_d]).
Softmax drops the max-shift (est bounded ~|6.5|, exp safe in f32). The
per-edge weighted slot-sum runs on the Tensor engine as identity-matmul
PSUM accumulation; DVE only does the est chain + one blast-multiply per
graph. LSTM stage unchanged (core-local via the scramble block structure).
"""

import os
import sys
import numpy as np
import ml_dtypes

BF16 = ml_dtypes.bfloat16

N = 2000
NPAD = 2048
T = 24
B = 2
NX = 8
H = 64
E = 32000
PL = 12
NB = 16          # dst blocks of 128 pi-positions
DUM = 2048       # dummy table row index
TROWS = 2049     # rows in the 6-graph table
LANES = 512      # table row width (bf16): 6*64 h | 6 es | pad
ESL = 384        # es lane base
HL = 768         # l1out row width (6 x 128-lane chunks, h at chunk base)
NEG = -1e9
LN_EPS = 1e-5
NCORES = 8
SEQ = 500        # LSTM sequences per core


def _ensure_hook():
    """Register the NTFF profile hook if the boot didn't (enables traces)."""
    try:
        try:
            import antenv.axon_hooks  # noqa: F401
        except ImportError:
            import types
            import antenv
            mod = types.ModuleType("antenv.axon_hooks")
            mod._hook = None

            def set_axon_ntff_profile_hook(h, _m=mod):
                _m._hook = h

            def get_axon_ntff_profile_hook(_m=mod):
                return _m._hook

            mod.set_axon_ntff_profile_hook = set_axon_ntff_profile_hook
            mod.get_axon_ntff_profile_hook = get_axon_ntff_profile_hook
            sys.modules["antenv.axon_hooks"] = mod
            antenv.axon_hooks = mod
        from antenv.axon_hooks import (get_axon_ntff_profile_hook,
                                       set_axon_ntff_profile_hook)
        if get_axon_ntff_profile_hook() is None:
            from trn_agent_boot.trn_boot import _ntff_profile_via_ctypes
            so = "/opt/axon/libaxon_pjrt.so"
            if os.path.exists(so):
                set_axon_ntff_profile_hook(_ntff_profile_via_ctypes(so))
    except Exception:
        pass


# ---------------------------------------------------------------------------
# Host-side prep
# ---------------------------------------------------------------------------

def fmt_idx(flat):
    flat = np.asarray(flat)
    assert len(flat) % 128 == 0
    fmt = flat.reshape(-1, 16).T.astype(np.int16)
    return np.tile(fmt, (8, 1))


def prep_structure(edge_src, edge_dst):
    src = np.asarray(edge_src, np.int64)
    dst = np.asarray(edge_dst, np.int64)
    deg = np.bincount(dst, minlength=N)
    pi = np.argsort(deg, kind="stable")
    pi_inv = np.empty(N, np.int64)
    pi_inv[pi] = np.arange(N)

    dpos = pi_inv[dst]
    order = np.argsort(dpos, kind="stable")
    src_s = src[order]
    dpos_s = dpos[order]
    cnt = np.bincount(dpos_s, minlength=NPAD)
    dmax = [int(max(1, cnt[b * 128:(b + 1) * 128].max())) for b in range(NB)]
    coloff = np.concatenate([[0], np.cumsum(dmax)]).astype(np.int64)
    ncol = int(coloff[-1])

    starts = np.concatenate([[0], np.cumsum(cnt)]).astype(np.int64)
    slot = np.full((ncol, 128), DUM, np.int64)
    b_of = dpos_s // 128
    p_of = dpos_s % 128
    jw = np.arange(E) - starts[dpos_s]
    slot[coloff[b_of] + jw, p_of] = pi_inv[src_s]
    eidx = fmt_idx(slot.reshape(-1))

    # LSTM gather rows: (t, local-seq nl) -> lstmtab row node_pi*6 + lg
    per_t = []
    for t in range(T):
        nl = np.arange(SEQ)
        flat_i = nl * T + t
        lg = flat_i // N
        lrow = pi_inv[flat_i % N] * 6 + lg
        lrow = np.concatenate([lrow, np.zeros(12, np.int64)])
        per_t.append(fmt_idx(lrow))  # [128, 32]
    lstmidx = np.ascontiguousarray(np.stack(per_t, axis=1))  # [128, 24, 32]

    return dict(pi=pi, pi_inv=pi_inv, dmax=dmax, coloff=coloff, ncol=ncol,
                eidx=eidx, lstmidx=lstmidx)


def prep_core(st, core):
    pi, pi_inv = st["pi"], st["pi_inv"]
    c = core % 4
    x2rows = np.full((6, NPAD), DUM, np.int64)
    sel = np.zeros((6, NPAD), np.int64)
    ppos = np.arange(N)
    for lg in range(6):
        t2 = 6 * c + lg
        flat_i = pi[ppos] * T + t2
        tp, npr = flat_i // N, flat_i % N
        # agtab row: [chunk(4 blocks), src-core, row-in-chunk]
        q = pi_inv[npr] // 512
        x2rows[lg, :N] = q * 2048 + (tp // 6) * 512 + pi_inv[npr] % 512
        sel[lg, :N] = tp % 6
    x2rows[:, N:] = 0
    x2idx = np.stack([fmt_idx(x2rows[lg]) for lg in range(6)], axis=1)
    # 5 predicated-select masks per graph (k=1..5); k=0 is the base copy.
    selmask = np.zeros((6, 5, NPAD), np.uint8)
    for lg in range(6):
        for k in range(1, 6):
            selmask[lg, k - 1] = (sel[lg] == k).astype(np.uint8)
    return dict(x2idx=np.ascontiguousarray(x2idx.astype(np.int16)),
                selmask=selmask)


def prep_weights(p):
    f32 = np.float32
    w = {}
    w["fcW"] = np.asarray(p["fc_W"], f32).astype(BF16)
    w["fcb"] = np.asarray(p["fc_b"], f32).reshape(64, 1)

    def wc(tag):
        W = np.asarray(p[f"{tag}_W"], f32)
        a_s = np.asarray(p[f"{tag}_asrc"], f32).reshape(64, 1)
        a_d = np.asarray(p[f"{tag}_adst"], f32).reshape(64, 1)
        return np.hstack([W, W @ a_s, W @ a_d]).astype(BF16)  # [64, 66]

    w["W1c"] = wc("g1")
    w["W2c"] = wc("g2")
    w["b1"] = np.asarray(p["g1_b"], f32).reshape(1, 64)
    w["b2"] = np.asarray(p["g2_b"], f32).reshape(1, 64)
    Wih = np.asarray(p["lstm_Wih"], f32)
    Whh = np.asarray(p["lstm_Whh"], f32)
    w["LL"] = np.vstack([Whh.T, Wih.T]).astype(BF16)  # K rows: 0:64=h, 64:128=x
    bihh = np.asarray(p["lstm_bih"], f32) + np.asarray(p["lstm_bhh"], f32)
    w["lstmb"] = np.ascontiguousarray(bihh.reshape(4, 64).T)  # [64,4] i,f,g,o
    w["lng"] = np.asarray(p["ln_g"], f32).reshape(64, 1)
    w["lnb"] = np.asarray(p["ln_b"], f32).reshape(64, 1)
    w["dW"] = np.asarray(p["dense_W"], f32)
    w["db"] = np.asarray(p["dense_b"], f32).reshape(1, 12)
    dum = np.zeros((1, LANES), f32)
    for g in range(6):
        dum[0, g * 65 + 64] = NEG
    w["dumrow"] = dum.astype(BF16)
    w["z384"] = np.zeros((1, HL), BF16)
    w["ident"] = np.eye(128, dtype=BF16)
    return w


def prep_xT(p, st, core):
    x = np.asarray(p["x"], np.float32)
    b, c = core // 4, core % 4
    pi = st["pi"]
    xT = np.zeros((NX, 6 * NPAD), np.float32)
    for lg in range(6):
        xT[:, lg * NPAD: lg * NPAD + N] = x[b, pi, 6 * c + lg, :].T
    return np.ascontiguousarray(xT.astype(BF16))


# ---------------------------------------------------------------------------
# Bass program
# ---------------------------------------------------------------------------

def build_program(meta):
    import concourse.bass as bass
    import concourse.mybir as mybir
    import concourse.tile as tile
    from concourse import bacc

    dt = mybir.dt
    Alu = mybir.AluOpType
    Act = mybir.ActivationFunctionType
    AX = mybir.AxisListType
    dmax, coloff, ncol = meta["dmax"], meta["coloff"], meta["ncol"]

    nc = bacc.Bacc("TRN2", target_bir_lowering=False, debug=False,
                   num_devices=NCORES)

    def din(name, shape, dty):
        return nc.dram_tensor(name, list(shape), dty, kind="ExternalInput")

    xT_d = din("xT", (NX, 6 * NPAD), dt.bfloat16)
    eidx_d = din("eidx", (128, ncol * 8), dt.int16)
    x2idx_d = din("x2idx", (128, 6, 128), dt.int16)
    lstmidx_d = din("lstmidx", (128, T, 32), dt.int16)
    selmask_d = din("selmask", (6, 5, NPAD), dt.uint8)
    fcW_d = din("fcW", (NX, 64), dt.bfloat16)
    fcb_d = din("fcb", (64, 1), dt.float32)
    W1c_d = din("W1c", (64, 66), dt.bfloat16)
    W2c_d = din("W2c", (64, 66), dt.bfloat16)
    b1_d = din("b1", (1, 64), dt.float32)
    b2_d = din("b2", (1, 64), dt.float32)
    LL_d = din("LL", (128, 256), dt.bfloat16)
    lstmb_d = din("lstmb", (64, 4), dt.float32)
    lng_d = din("lng", (64, 1), dt.float32)
    lnb_d = din("lnb", (64, 1), dt.float32)
    dW_d = din("dW", (64, 12), dt.float32)
    db_d = din("db", (1, 12), dt.float32)
    dum_d = din("dumrow", (1, LANES), dt.bfloat16)
    z384_d = din("z384", (1, HL), dt.bfloat16)
    ident_d = din("ident", (128, 128), dt.bfloat16)
    out_d = nc.dram_tensor("out", [SEQ, 12], dt.float32, kind="ExternalOutput")

    def bcast_ap(dram_t, parts, inner):
        """Partition-broadcast read AP of a DRAM region."""
        ap = dram_t if isinstance(dram_t, bass.AP) else dram_t.ap()
        return bass.AP(tensor=ap.tensor, offset=ap.offset,
                       ap=[[0, parts]] + [list(x) for x in ap.ap[1:]])

    def blast(ap_, n):
        """Append a step-0 dim of size n to an AP (free-dim broadcast)."""
        return bass.AP(tensor=ap_.tensor, offset=ap_.offset,
                       ap=list(ap_.ap) + [[0, n]])

    def midblast(ap_, n):
        """Insert a step-0 dim of size n before the last dim of an AP."""
        return bass.AP(tensor=ap_.tensor, offset=ap_.offset,
                       ap=list(ap_.ap[:-1]) + [[0, n], list(ap_.ap[-1])])

    with tile.TileContext(nc) as tc:
        import contextlib
        ctx = contextlib.ExitStack()
        consts = ctx.enter_context(tc.tile_pool(name="consts", bufs=1))
        dramp = ctx.enter_context(tc.tile_pool(name="dramp", bufs=1,
                                               space="DRAM"))
        rows_p = ctx.enter_context(tc.tile_pool(name="rows", bufs=3))
        msg_p = ctx.enter_context(tc.tile_pool(name="msg", bufs=2))
        wk_p = ctx.enter_context(tc.tile_pool(name="wk", bufs=4))
        wm_p = ctx.enter_context(tc.tile_pool(name="wm", bufs=3))
        edt_p = ctx.enter_context(tc.tile_pool(name="edt", bufs=1))

        # ---- constants into SBUF ----
        def load(name, shape, dty, src_ap):
            t = consts.tile(shape, dty, name=name)
            nc.sync.dma_start(out=t[:], in_=src_ap)
            return t

        eidx_s = load("eidx_s", [128, ncol * 8], dt.int16, eidx_d[:])
        x2idx_s = load("x2idx_s", [128, 6, 128], dt.int16, x2idx_d[:])
        lidx_s = load("lidx_s", [128, T, 32], dt.int16, lstmidx_d[:])
        fcW_s = load("fcW_s", [NX, 64], dt.bfloat16, fcW_d[:])
        fcb_s = load("fcb_s", [64, 1], dt.float32, fcb_d[:])
        W1c_s = load("W1c_s", [64, 66], dt.bfloat16, W1c_d[:])
        W2c_s = load("W2c_s", [64, 66], dt.bfloat16, W2c_d[:])
        LL_s = load("LL_s", [128, 256], dt.bfloat16, LL_d[:])
        dW_s = load("dW_s", [64, 12], dt.float32, dW_d[:])
        lstmb_s = load("lstmb_s", [64, 4], dt.float32, lstmb_d[:])
        lng_s = load("lng_s", [64, 1], dt.float32, lng_d[:])
        lnb_s = load("lnb_s", [64, 1], dt.float32, lnb_d[:])
        ident_s = load("ident_s", [128, 128], dt.bfloat16, ident_d[:])
        bR = {}
        for nm, d_ in (("b1", b1_d), ("b2", b2_d)):
            t = consts.tile([128, 64], dt.float32, name=nm + "R")
            nc.gpsimd.dma_start(out=t[:], in_=bcast_ap(d_, 128, 64))
            bR[nm] = t
        dbR = consts.tile([128, 12], dt.float32, name="dbR")
        nc.gpsimd.dma_start(out=dbR[:], in_=bcast_ap(db_d, 128, 12))
        onesrow = consts.tile([1, 64], dt.float32, name="onesrow")
        nc.vector.memset(onesrow[:], 1.0)
        onescol = consts.tile([128, 1], dt.float32, name="onescol")
        nc.vector.memset(onescol[:], 1.0)
        epsT = consts.tile([1, 1], dt.float32, name="epsT")
        nc.vector.memset(epsT[:], LN_EPS)

        # ---- DRAM tables ----
        htab = {1: dramp.tile([TROWS, LANES], dt.bfloat16, name="htab1"),
                2: dramp.tile([TROWS, LANES], dt.bfloat16, name="htab2")}
        # l1out in 4 chunks of 4 dst-blocks so each AllGather chunk can
        # launch as soon as its blocks are done (slice-level dep via tiles).
        l1out = [dramp.tile([512, HL], dt.bfloat16, name=f"l1out{q}")
                 for q in range(4)]
        agtab = dramp.tile([4 * 2048, HL], dt.bfloat16, name="agtab")
        lstmtab = dramp.tile([TROWS, 6, 128], dt.bfloat16, name="lstmtab")

        edt_all = {1: edt_p.tile([128, NB, 6], dt.float32, name="edt1"),
                   2: edt_p.tile([128, NB, 6], dt.float32, name="edt2")}

        # ---- FC: hfcT bf16 = fcW.T @ xT + fcb, split in 4 chunk-group
        # tiles (chn-group q holds cols g*512 + (chn%4)*128) so the L1
        # table build can start on group 0 while FC fills groups 1-3.
        fc_ctx = contextlib.ExitStack()
        hfc_p = fc_ctx.enter_context(tc.tile_pool(name="hfc", bufs=1))
        hfcQ = [hfc_p.tile([64, 6 * 512], dt.bfloat16, name=f"hfcT{q}")
                for q in range(4)]
        with tc.tile_pool(name="xtp", bufs=1) as xt_p, \
             tc.tile_pool(name="ps_fc", bufs=2, space="PSUM") as ps_fc:
            xT_s = xt_p.tile([NX, 6 * NPAD], dt.bfloat16, name="xT_s")
            nc.sync.dma_start(out=xT_s[:], in_=xT_d[:])
            for q in range(4):
                for g in range(6):
                    sl = slice(g * NPAD + q * 512, g * NPAD + (q + 1) * 512)
                    ps = ps_fc.tile([64, 512], dt.float32, space="PSUM",
                                    tag="psfc")
                    nc.tensor.matmul(ps[:], lhsT=fcW_s[:], rhs=xT_s[:, sl],
                                     start=True, stop=True)
                    nc.vector.tensor_scalar_add(
                        hfcQ[q][:, g * 512:(g + 1) * 512], ps[:], fcb_s[:])

        def table_build(layer, lhsT_of):
            """Build the 6-graph h-table + edt for one layer."""
            Wc = W1c_s if layer == 1 else W2c_s
            edt = edt_all[layer]
            for chn in range(NB):
                rowt = rows_p.tile([128, LANES], dt.bfloat16, tag="rowt")
                for g in range(6):
                    ps = ps_h.tile([128, 66], dt.float32, space="PSUM",
                                   tag="psh")
                    nc.tensor.matmul(ps[:], lhsT=lhsT_of(g, chn), rhs=Wc[:],
                                     start=True, stop=True)
                    nc.scalar.activation(rowt[:, g * 65:g * 65 + 65],
                                         ps[:, 0:65], Act.Copy)
                    nc.vector.tensor_copy(edt[:, chn, g:g + 1],
                                          ps[:, 65:66])
                nc.sync.dma_start(
                    out=htab[layer][chn * 128:(chn + 1) * 128, 0:390],
                    in_=rowt[:, 0:390])
            nc.sync.dma_start(out=htab[layer][DUM:DUM + 1, :], in_=dum_d[:])

        def sparse_phase(layer, post_block=None):
            """Per-block gather + GAT softmax/aggregate, all 6 graphs."""
            bias = bR["b1"] if layer == 1 else bR["b2"]
            edt = edt_all[layer]
            for b in reversed(range(NB)):
                D = dmax[b]
                msg = msg_p.tile([128, D, LANES], dt.bfloat16, tag="msg")
                nc.gpsimd.dma_gather(
                    msg[:], htab[layer],
                    eidx_s[:, 8 * int(coloff[b]): 8 * int(coloff[b] + D)],
                    128 * D, 128 * D, LANES, single_packet=False)
                # est [128, D, 6] = es(src) + ed(dst)
                est = wk_p.tile([128, D, 6], dt.float32, tag="est")
                edt_b = bass.AP(tensor=edt[:].tensor,
                                offset=edt[:].offset + b * 6,
                                ap=[list(edt[:].ap[0]), [0, D], [1, 6]])
                es_ap = bass.AP(tensor=msg[:].tensor,
                                offset=msg[:].offset + 64,
                                ap=[list(msg[:].ap[0]), [LANES, D], [65, 6]])
                nc.vector.tensor_tensor(out=est[:], in0=es_ap,
                                        in1=edt_b, op=Alu.add)
                estL = wk_p.tile([128, D * 6], dt.float32, tag="estL")
                ef = est[:].rearrange("p d g -> p (d g)")
                nc.vector.scalar_tensor_tensor(out=estL[:], in0=ef,
                                               scalar=0.2, in1=ef,
                                               op0=Alu.mult, op1=Alu.max)
                wbf = wk_p.tile([128, D, 6], dt.bfloat16, tag="wbf")
                nc.scalar.activation(
                    wbf[:].rearrange("p d g -> p (d g)"), estL[:], Act.Exp)
                den = wk_p.tile([128, 6], dt.float32, tag="den")
                nc.vector.tensor_reduce(den[:],
                                        wbf[:].rearrange("p d g -> p g d"),
                                        axis=AX.X, op=Alu.add)
                rcol = wk_p.tile([128, 6], dt.float32, tag="rcol")
                nc.vector.reciprocal(rcol[:], den[:])
                psum6 = ps_agg.tile([128, 6, 64], dt.float32, space="PSUM",
                                    tag="psagg")
                for g in range(6):
                    wm = wm_p.tile([128, D, 64], dt.bfloat16, tag="wm")
                    wbf_g = bass.AP(tensor=wbf[:].tensor,
                                    offset=wbf[:].offset + g,
                                    ap=[list(wbf[:].ap[0]), [6, D], [0, 64]])
                    nc.vector.tensor_tensor(out=wm[:],
                                            in0=msg[:, :, g * 65:g * 65 + 64],
                                            in1=wbf_g, op=Alu.mult)
                    for c in range(D):
                        nc.tensor.matmul(psum6[:, g, :], lhsT=ident_s[:],
                                         rhs=wm[:, c, :],
                                         start=(c == 0), stop=(c == D - 1))
                # normalize + bias over all 6 graphs at once
                gout = wk_p.tile([128, 6, 64], dt.float32, tag="gout")
                rcol_mid = bass.AP(tensor=rcol[:].tensor,
                                   offset=rcol[:].offset,
                                   ap=[list(rcol[:].ap[0]), [1, 6], [0, 64]])
                nc.vector.tensor_tensor(out=gout[:], in0=psum6[:],
                                        in1=rcol_mid, op=Alu.mult)
                bias_b = bass.AP(tensor=bias[:].tensor,
                                 offset=bias[:].offset,
                                 ap=[list(bias[:].ap[0]), [0, 6], [1, 64]])
                nc.vector.tensor_tensor(out=gout[:], in0=gout[:],
                                        in1=bias_b, op=Alu.add)
                if layer == 1:
                    outrow = rows_p.tile([128, 6, 128], dt.bfloat16,
                                         tag="orow")
                    nc.scalar.activation(outrow[:, :, 0:64], gout[:],
                                         Act.Gelu)
                    nc.sync.dma_start(
                        out=l1out[b // 4][(b % 4) * 128:(b % 4 + 1) * 128, :],
                        in_=outrow[:].rearrange("p g l -> p (g l)"))
                else:
                    lrow6 = rows_p.tile([128, 6, 128], dt.bfloat16,
                                        tag="lrow6")
                    nc.vector.memset(lrow6[:, :, 0:64], 0.0)
                    nc.scalar.activation(lrow6[:, :, 64:128], gout[:],
                                         Act.Gelu)
                    nc.sync.dma_start(
                        out=lstmtab[b * 128:(b + 1) * 128, :, :],
                        in_=lrow6[:])
                if post_block is not None:
                    post_block(b)

        gcn_ctx = contextlib.ExitStack()
        ps_h = gcn_ctx.enter_context(tc.tile_pool(name="ps_h", bufs=2,
                                                  space="PSUM"))
        ps_agg = gcn_ctx.enter_context(tc.tile_pool(name="ps_agg", bufs=2,
                                                    space="PSUM"))

        # ---- layer 1 (AllGather chunks fire as their 4 blocks finish) ----
        def l1_lhsT(g, chn):
            base = g * 512 + (chn % 4) * 128
            return hfcQ[chn // 4][:, base: base + 128]

        def l1_post(b):
            if b % 4 == 0:
                q = b // 4
                nc.gpsimd.collective_compute(
                    "AllGather", Alu.bypass,
                    replica_groups=[[0, 1, 2, 3], [4, 5, 6, 7]],
                    ins=[l1out[q][:].opt()],
                    outs=[agtab[q * 2048:(q + 1) * 2048, :].opt()])

        table_build(1, l1_lhsT)
        sparse_phase(1, post_block=l1_post)
        fc_ctx.close()

        # ---- layer 2 input: gather + 6-way select ----
        l2_ctx = contextlib.ExitStack()
        x2_p = l2_ctx.enter_context(tc.tile_pool(name="x2", bufs=2))
        x2s_p = l2_ctx.enter_context(tc.tile_pool(name="x2s", bufs=1))
        mask_p = l2_ctx.enter_context(tc.tile_pool(name="mask", bufs=2))
        x2sel = []
        for g in range(6):
            msk = mask_p.tile([64, 5, NPAD], dt.uint8, tag="msk")
            nc.gpsimd.dma_start(
                out=msk[:],
                in_=bass.AP(tensor=selmask_d.ap().tensor,
                            offset=g * 5 * NPAD,
                            ap=[[0, 64], [NPAD, 5], [1, NPAD]]))
            xs = x2s_p.tile([64, NPAD], dt.bfloat16, name=f"xs{g}")
            for hh in range(2):
                xg = x2_p.tile([128, 6, 1024], dt.bfloat16, tag=f"xg{hh}")
                nc.gpsimd.dma_gather(
                    xg[:], agtab[:],
                    x2idx_s[:, g, hh * 64:(hh + 1) * 64],
                    1024, 1024, HL, transpose=True,
                    single_packet=False)
                sl = slice(hh * 1024, (hh + 1) * 1024)
                nc.vector.tensor_copy(xs[:, sl], xg[0:64, 0, :])
                for k in range(1, 6):
                    nc.vector.copy_predicated(
                        xs[:, sl], msk[:, k - 1, sl], xg[0:64, k, :])
            x2sel.append(xs)

        # ---- layer 2 ----
        def l2_lhsT(g, chn):
            return x2sel[g][:, chn * 128:(chn + 1) * 128]
        table_build(2, l2_lhsT)
        sparse_phase(2)
        l2_ctx.close()
        gcn_ctx.close()

        # ---- LSTM ----
        lst_ctx = contextlib.ExitStack()
        big = lst_ctx.enter_context(tc.tile_pool(name="big", bufs=1))
        lst_p = lst_ctx.enter_context(tc.tile_pool(name="lst", bufs=2))
        ps_misc = lst_ctx.enter_context(tc.tile_pool(name="ps_misc", bufs=1,
                                                     space="PSUM"))
        ps_z = lst_ctx.enter_context(tc.tile_pool(name="ps_z", bufs=4,
                                                  space="PSUM"))
        lt = lstmtab[:]
        lt_flat = bass.AP(tensor=lt.tensor, offset=lt.offset,
                          ap=[[128, TROWS * 6], [1, 128]])
        XTs = []
        for tg in range(T):
            XT = big.tile([128, 1, 512], dt.bfloat16, name=f"XT{tg}")
            nc.gpsimd.dma_gather(XT[:], lt_flat,
                                 lidx_s[:, tg, :], 512, 500, 128,
                                 transpose=True, single_packet=False)
            XTs.append(XT)
        stacked = big.tile([128, SEQ], dt.bfloat16, name="stacked")
        cT = big.tile([64, SEQ], dt.float32, name="cT")
        h23 = big.tile([64, SEQ], dt.float32, name="h23")
        nc.vector.memset(stacked[0:64, :], 0.0)
        nc.vector.memset(cT[:], 0.0)
        for t in range(T):
            nc.vector.tensor_copy(stacked[64:128, :], XTs[t][64:128, 0, 0:SEQ])
            zs = []
            for g in range(4):
                ps = ps_z.tile([64, SEQ], dt.float32, space="PSUM",
                               tag="pslstm", name=f"z{g}")
                nc.tensor.matmul(ps[:], lhsT=LL_s[:, g * 64:(g + 1) * 64],
                                 rhs=stacked[:], start=True, stop=True)
                zs.append(ps)
            gates = [None] * 4
            for g, fn in ((0, Act.Sigmoid), (1, Act.Sigmoid),
                          (3, Act.Sigmoid), (2, Act.Tanh)):
                gt = lst_p.tile([64, SEQ], dt.float32, tag=f"gate{g}",
                                name=f"gate{g}")
                nc.scalar.activation(gt[:], zs[g][:], fn,
                                     bias=lstmb_s[:, g:g + 1], scale=1.0)
                gates[g] = gt
            ig = lst_p.tile([64, SEQ], dt.float32, tag="ig")
            nc.vector.tensor_mul(ig[:], gates[0][:], gates[2][:])
            nc.vector.tensor_mul(cT[:], gates[1][:], cT[:])
            nc.vector.tensor_add(cT[:], cT[:], ig[:])
            th = lst_p.tile([64, SEQ], dt.float32, tag="th")
            nc.scalar.activation(th[:], cT[:], Act.Tanh)
            if t < T - 1:
                nc.vector.tensor_mul(stacked[0:64, :], gates[3][:], th[:])
            else:
                nc.vector.tensor_mul(h23[:], gates[3][:], th[:])

        # ---- LayerNorm (transposed; stats via ones-matmuls) ----
        ps_mu = ps_misc.tile([1, SEQ], dt.float32, space="PSUM", tag="psmisc",
                             name="ps_mu")
        nc.tensor.matmul(ps_mu[:], lhsT=onescol[0:64, :], rhs=h23[:],
                         start=True, stop=True)
        mu_sb = lst_p.tile([1, SEQ], dt.float32, tag="mu")
        nc.scalar.activation(mu_sb[:], ps_mu[:], Act.Copy, scale=1.0 / 64)
        ps_mub = ps_misc.tile([64, SEQ], dt.float32, space="PSUM",
                              tag="psb500", name="ps_mub")
        nc.tensor.matmul(ps_mub[:], lhsT=onesrow[:], rhs=mu_sb[:],
                         start=True, stop=True)
        dtl = lst_p.tile([64, SEQ], dt.float32, tag="dtl")
        nc.vector.tensor_sub(dtl[:], h23[:], ps_mub[:])
        sq = lst_p.tile([64, SEQ], dt.float32, tag="sq")
        nc.vector.tensor_mul(sq[:], dtl[:], dtl[:])
        ps_var = ps_misc.tile([1, SEQ], dt.float32, space="PSUM", tag="psmisc",
                              name="ps_var")
        nc.tensor.matmul(ps_var[:], lhsT=onescol[0:64, :], rhs=sq[:],
                         start=True, stop=True)
        sd_sb = lst_p.tile([1, SEQ], dt.float32, tag="sd")
        nc.scalar.activation(sd_sb[:], ps_var[:], Act.Sqrt, bias=epsT[:],
                             scale=1.0 / 64)
        rstd = lst_p.tile([1, SEQ], dt.float32, tag="rstd")
        nc.vector.reciprocal(rstd[:], sd_sb[:])
        ps_rb = ps_misc.tile([64, SEQ], dt.float32, space="PSUM",
                             tag="psb500", name="ps_rb")
        nc.tensor.matmul(ps_rb[:], lhsT=onesrow[:], rhs=rstd[:],
                         start=True, stop=True)
        hn = lst_p.tile([64, SEQ], dt.float32, tag="hn")
        nc.vector.tensor_mul(hn[:], dtl[:], ps_rb[:])
        nc.vector.tensor_scalar(out=hn[:], in0=hn[:],
                                scalar1=lng_s[:], scalar2=lnb_s[:],
                                op0=Alu.mult, op1=Alu.add)
        # ---- dense ----
        for q in range(4):
            cs = min(128, SEQ - q * 128)
            psd = ps_misc.tile([128, 12], dt.float32, space="PSUM",
                               tag="psmisc", name="psd")
            nc.tensor.matmul(psd[0:cs, :],
                             lhsT=hn[:, q * 128:q * 128 + cs],
                             rhs=dW_s[:], start=True, stop=True)
            ot = lst_p.tile([128, 12], dt.float32, tag="ot")
            nc.vector.tensor_add(ot[0:cs, :], psd[0:cs, :], dbR[0:cs, :])
            nc.sync.dma_start(out=out_d[q * 128:q * 128 + cs, :],
                              in_=ot[0:cs, :])
        lst_ctx.close()
        ctx.close()

    nc.compile()
    return nc


# ---------------------------------------------------------------------------
# Runner
# ---------------------------------------------------------------------------

_CACHE = {}


def _get_program_and_maps(inputs):
    st = prep_structure(inputs["edge_src"], inputs["edge_dst"])
    w = prep_weights(inputs)
    meta = dict(dmax=st["dmax"], coloff=st["coloff"], ncol=st["ncol"])
    key = ("prog", tuple(st["dmax"]))
    if key not in _CACHE:
        _CACHE[key] = build_program(meta)
    nc = _CACHE[key]

    shared = dict(eidx=st["eidx"], lstmidx=st["lstmidx"], fcW=w["fcW"],
                  fcb=w["fcb"], W1c=w["W1c"], W2c=w["W2c"],
                  b1=w["b1"], b2=w["b2"], LL=w["LL"], lstmb=w["lstmb"],
                  lng=w["lng"], lnb=w["lnb"], dW=w["dW"], db=w["db"],
                  dumrow=w["dumrow"], z384=w["z384"], ident=w["ident"])
    in_maps = []
    for core in range(NCORES):
        pc = prep_core(st, core)
        m = dict(shared)
        m["xT"] = prep_xT(inputs, st, core)
        m["x2idx"] = pc["x2idx"]
        m["selmask"] = pc["selmask"]
        in_maps.append(m)
    return nc, in_maps


def run_on_hw(inputs, trace=False):
    _ensure_hook()
    from concourse.bass_utils import run_bass_kernel_spmd
    nc, in_maps = _get_program_and_maps(inputs)
    res = run_bass_kernel_spmd(nc, in_maps, list(range(NCORES)), trace=trace)
    out_full = np.zeros((B, N, PL, 1), np.float32)
    for core in range(NCORES):
        bb, c = core // 4, core % 4
        out_full[bb, 500 * c:500 * (c + 1), :, 0] = res.results[core]["out"]
    return out_full, res


def kernel(x, edge_src, edge_dst, fc_W, fc_b,
           g1_W, g1_b, g1_asrc, g1_adst,
           g2_W, g2_b, g2_asrc, g2_adst,
           lstm_Wih, lstm_Whh, lstm_bih, lstm_bhh,
           ln_g, ln_b, dense_W, dense_b):
    inputs = dict(x=x, edge_src=edge_src, edge_dst=edge_dst, fc_W=fc_W,
                  fc_b=fc_b, g1_W=g1_W, g1_b=g1_b, g1_asrc=g1_asrc,
                  g1_adst=g1_adst, g2_W=g2_W, g2_b=g2_b, g2_asrc=g2_asrc,
                  g2_adst=g2_adst, lstm_Wih=lstm_Wih, lstm_Whh=lstm_Whh,
                  lstm_bih=lstm_bih, lstm_bhh=lstm_bhh, ln_g=ln_g, ln_b=ln_b,
                  dense_W=dense_W, dense_b=dense_b)
    out, _ = run_on_hw(inputs, trace=False)
    return out
